# revision 28
# baseline (speedup 1.0000x reference)
"""Trainium2 Bass kernel: pre-LN transformer block (B=4, T=2048, E=1024, H=16, FFN=100).

Sharding (8 NeuronCores): core 2b+g handles batch b, head-group g (8 of 16 heads,
i.e. a 512-wide slice of the QKV output dim / proj input dim).  Both cores of a
pair compute attention + proj partials for all 2048 tokens of their batch; two
per-pair ReduceScatters (bf16, Shared outputs) combine the partials and hand
each core half the tokens, on which it runs LN2 + FFN and writes its
[1024, 1024] output shard.

SPMD notes: all 8 cores run one program; per-core behavior differs only via
input data.  The residual is fed as x/2 on both pair members (summed back to x
by the reduce); LN1 uses eps/4 so layernorm(x/2, eps/4) == layernorm(x, eps)
exactly.  b_proj/2 is folded host-side into the proj residual copy of x (xb),
and b2 is folded into the FFN second matmul as an extra input row.

Attention layout: scores are computed transposed, S^T[t_k, t_q] = k^T.T @ q^T,
with q^T/k^T in [head_dim, token] layout (from PE-transposed LN output, all in
bf16 so transposes run at 1 cycle/row).  Softmax runs without max subtraction
(logits are ~N(0, 0.25), safe in fp32): exp on ScalarE straight out of PSUM
with the 1/sqrt(E) scale folded in.  Causality is exploited at 128-column
granularity: for the diagonal t_k tile at offset m only the suffix columns
[m*128:) are computed/exp'd, and a single [128,128] lower-triangle mask fixes
the diagonal block.  The denominator comes from an extra ones-column appended
to V; its reciprocal row is broadcast across partitions 64:128 of the same AV
PSUM bank via a K=1 ones matmul (fully on-chip - no DRAM bounce), and one DVE
multiply produces the normalized attention output.
"""

from contextlib import ExitStack

import numpy as np
import ml_dtypes

import concourse.bass as bass
import concourse.mybir as mybir
import concourse.tile as tile
from concourse.bass_utils import run_bass_kernel_spmd
from concourse.vector_clock import ScopedClock


class SplitDrainTC(tile.TileContext):
    """Works around a walrus codegen limit: an SP CTRL instruction may carry
    only one sync wait, so the kernel-tail drain's waits are split onto
    preceding single-wait nops."""

    def _drain_and_barrier(self, tick_clock, wait_clock):
        probe = self.nc.sync.nop(nofuse=True)
        wait_clock.add_sem_waits(
            probe.ins, ScopedClock({None: tick_clock.global_clock})
        )
        si = probe.ins.sync_info
        waits = list(si.on_wait) if si is not None else []
        if len(waits) > 1:
            si.on_wait = [waits[0]]
            for w in waits[1:]:
                n2 = self.nc.sync.nop(nofuse=True)
                n2.ins.sync_info = mybir.SyncInfo(on_wait=[w], on_update=[])
        self.nc.sync.drain()
        self.nc.all_engine_barrier()
        popped = self.nc._tile_sem_poison_stack.pop()
        assert popped is self._sem_poison
        self.nc.clear_and_free_semaphores(list(self.sems.allocated().values()))
        self.nc.all_engine_barrier()

B, T, E, H, HS, FFN = 4, 2048, 1024, 16, 64, 100
EPS = 1e-5
NCORE = 8
TC = 512            # token chunk
NTC = T // TC       # 4
TS = 128            # token subtile
NSUB = TC // TS     # 4
ET = 128            # embed tile
NET = E // ET       # 8
DSL = E // 2        # per-core qkv output slice (8 heads * 64)
NDT = DSL // 128    # 4 d-tiles (2 heads each)
HPC = H // 2        # 8 heads per core
SCALE = float(E) ** -0.5
PAIRS = [[0, 1], [2, 3], [4, 5], [6, 7]]

MM_MODE = "bf16"    # "bf16" | "f32"
AF = mybir.ActivationFunctionType


def _mdt(mode):
    return mybir.dt.bfloat16 if mode == "bf16" else mybir.dt.float32


def _np_mdt(mode):
    return ml_dtypes.bfloat16 if mode == "bf16" else np.float32


def build(mode=MM_MODE):
    f32 = mybir.dt.float32
    mdt = _mdt(mode)

    nc = bass.Bass(num_devices=NCORE)

    io = {}

    def param(name, shape, dtype):
        io[name] = nc.declare_dram_parameter(name, shape, dtype, isOutput=False)

    param("xr", [T, E], f32)           # x/2 (LN1 input)
    param("xb", [T, E], f32)           # x/2 + b_proj/2 (proj residual)
    param("wq", [E, DSL], mdt)
    param("wk", [E, DSL], mdt)
    param("wv", [E, DSL], mdt)
    param("wp", [DSL, E], mdt)
    param("w1", [E, FFN], mdt)
    param("w2e", [FFN + 1, E], mdt)    # w2 with b2 as the extra last row
    param("b1", [FFN, 1], f32)
    param("ln1g", [E, 1], f32)
    param("ln1b", [E, 1], f32)
    param("ln2g", [E, 1], f32)
    param("ln2b", [E, 1], f32)
    param("mask", [TS, TS], mdt)       # lower-triangle (t_k <= t_q) block mask
    param("ident", [TS, TS], mdt)
    io["out"] = nc.declare_dram_parameter("out", [T // 2, E], f32, isOutput=True)

    with SplitDrainTC(nc) as tc:
        with ExitStack() as ctx:
            _build_tile(ctx, tc, nc, mode, mdt, f32, io)
    _split_waits(nc)
    return nc


def _split_waits(nc, maxw=1):
    """walrus codegen accepts a limited number of sync waits per instruction;
    move the excess onto same-engine NoOps inserted just before."""
    import bass_rust
    n = 0
    for f in nc.m.functions:
        for b in f.blocks:
            new = []
            for inst in b.instructions:
                si = inst.sync_info
                # fixed-length ISA instructions can't carry waits at all
                cap = 0 if isinstance(inst, bass_rust.InstISA) else maxw
                if si is not None and len(si.on_wait) > cap:
                    waits = list(si.on_wait)
                    keep = waits[-cap:] if cap else []
                    excess = waits[:-cap] if cap else waits
                    for w in excess:
                        nop = mybir.InstNoOp(
                            name=f"{inst.name}-wsplit{n}", engine=inst.engine
                        )
                        nop.bass_nofuse = True
                        n += 1
                        nop.sync_info = mybir.SyncInfo(
                            on_wait=[w], on_update=[]
                        )
                        new.append(nop)
                    si.on_wait = keep
                new.append(inst)
            if n:
                b.instructions = new


def _build_tile(ctx, tc, nc, mode, mdt, f32, io):
    xr, xb, out = io["xr"], io["xb"], io["out"]

    def pool(name, bufs, space="SBUF"):
        return ctx.enter_context(tc.tile_pool(name=name, bufs=bufs, space=space))

    # ---- internal DRAM ----
    dram = pool("dram", 1, space="DRAM")
    ar_in = dram.tile([T, E], mdt, name="ar_in")
    rs_out = [
        dram.tile([TC, E], mdt, name="rs0_out"),
        dram.tile([TC, E], mdt, name="rs1_out"),
    ]

    # ---- persistent SBUF: weights & constants (DMAs on the idle Pool
    # sequencer; its DGE config time is ~25ns vs SP's ~565ns) ----
    wpool = pool("weights", 1)
    wq_sb = wpool.tile([ET, NET, DSL], mdt, name="wq_sb")
    wk_sb = wpool.tile([ET, NET, DSL], mdt, name="wk_sb")
    wv_sb = wpool.tile([ET, NET, DSL], mdt, name="wv_sb")
    # big weights go on the (otherwise idle) SP queue so the gpsimd queue can
    # start streaming x tiles immediately
    nc.sync.dma_start(out=wq_sb, in_=io["wq"].rearrange("(k p) d -> p k d", p=ET))
    nc.sync.dma_start(out=wk_sb, in_=io["wk"].rearrange("(k p) d -> p k d", p=ET))
    nc.sync.dma_start(out=wv_sb, in_=io["wv"].rearrange("(k p) d -> p k d", p=ET))
    wp_sb = wpool.tile([128, NDT, E], mdt, name="wp_sb")
    nc.sync.dma_start(out=wp_sb, in_=io["wp"].rearrange("(k p) d -> p k d", p=128))
    ones64b = wpool.tile([1, HS], mdt, name="ones64b")
    nc.vector.memset(ones64b, 1.0)
    w1_sb = wpool.tile([ET, NET, FFN], mdt, name="w1_sb")
    nc.sync.dma_start(out=w1_sb, in_=io["w1"].rearrange("(k p) d -> p k d", p=ET))
    w2_sb = wpool.tile([FFN + 1, E], mdt, name="w2_sb")
    nc.sync.dma_start(out=w2_sb, in_=io["w2e"][:])
    b1_sb = wpool.tile([FFN, 1], f32, name="b1_sb")
    nc.sync.dma_start(out=b1_sb, in_=io["b1"][:])
    ln_sb = {}
    for nm in ("ln1g", "ln1b", "ln2g", "ln2b"):
        ln_sb[nm] = wpool.tile([ET, NET, 1], f32, name=nm + "_sb")
        nc.gpsimd.dma_start(
            out=ln_sb[nm], in_=io[nm].rearrange("(k p) o -> p k o", p=ET)
        )
    mask_sb = wpool.tile([TS, TS], mdt, name="mask_sb")
    nc.gpsimd.dma_start(out=mask_sb, in_=io["mask"][:])
    id_sb = wpool.tile([TS, TS], mdt, name="id_sb")
    nc.gpsimd.dma_start(out=id_sb, in_=io["ident"][:])
    eps1_sb = wpool.tile([128, 1], f32, name="eps1_sb")
    nc.vector.memset(eps1_sb, EPS / 4.0)  # LN1 runs on x/2
    eps2_sb = wpool.tile([128, 1], f32, name="eps2_sb")
    nc.vector.memset(eps2_sb, EPS)

    # ---- persistent SBUF: per-chunk K^T, V(+ones), Q^T ----
    kv = pool("kv", 1)
    kT_c = [kv.tile([128, NDT, TC], mdt, name=f"kT{c}") for c in range(NTC)]
    vt_c = [kv.tile([128, NSUB, HPC, HS + 1], mdt, name=f"vt{c}")
            for c in range(NTC)]
    qT_c = [kv.tile([128, NDT, TC], mdt, name=f"qT{c}") for c in range(NTC)]

    # ---- working pools ----
    xt_pool = pool("xt", 5)
    h_pool = pool("h", 6)
    mv_pool = pool("mv", 3)
    hT_pool = pool("hT", 2)
    pt_pool = pool("pt", 4)
    avs_pool = pool("avs", 4)
    rec_pool = pool("rec", 2)
    attT_pool = pool("attT", 8)   # att(0) and att(2) outputs both live
    xb_pool = pool("xbp", 2)
    part_pool = pool("part", 3)
    x2_pool = pool("x2", 4)
    f1_pool = pool("f1", 2)
    out_pool = pool("outp", 2)
    ps_mm = pool("ps_mm", 2, space="PSUM")
    ps_sc = pool("ps_sc", 4, space="PSUM")
    ps_av = pool("ps_av", 2, space="PSUM")

    def layer_norm_chunk(x_ts, eps_tile, out_ts):
        """out_ts[s] (bf16) = (x_ts[s] - mean) * rsqrt(var + eps), with the
        4 subtiles' stats batched so Sqrt costs one Act instruction (one
        activation-table region instead of four)."""
        n = len(x_ts)
        mv = mv_pool.tile([128, n, 2], f32, name="mv")
        for s, x_t in enumerate(x_ts):
            stats = mv_pool.tile(
                [128, 2, nc.vector.BN_STATS_DIM], f32, name="stats"
            )
            xg = x_t.rearrange("p (u q) -> p u q", u=2)
            for u in range(2):
                nc.vector.bn_stats(out=stats[:, u, :], in_=xg[:, u, :])
            nc.vector.bn_aggr(out=mv[:, s, :], in_=stats)
        rstd = mv_pool.tile([128, n], f32, name="rstd")
        nc.scalar.activation(
            out=rstd, in_=mv[:, :, 1], func=AF.Sqrt, bias=eps_tile, scale=1.0
        )
        nc.vector.reciprocal(out=rstd, in_=rstd)
        for s, x_t in enumerate(x_ts):
            nc.vector.tensor_scalar(
                out=out_ts[s], in0=x_t, scalar1=mv[:, s, 0:1],
                scalar2=rstd[:, s:s + 1],
                op0=mybir.AluOpType.subtract, op1=mybir.AluOpType.mult,
            )

    def transpose_cast(h_ts, g_sb, b_sb, hT):
        """PE-transpose 4 subtiles of h [128, E] (bf16) into hT[:, k, :],
        batching the 4 128x128 transposes of one k-tile into one PSUM tile so
        the layernorm scale/bias fold costs one DVE op per [128, 512]."""
        for k in range(NET):
            tp = ps_mm.tile([TS, TC], mdt, name="tp", tag="mm")
            for s in range(NSUB):
                nc.tensor.transpose(
                    tp[:, s * TS:(s + 1) * TS],
                    h_ts[s][:, k * ET:(k + 1) * ET], id_sb,
                )
            nc.vector.tensor_scalar(
                out=hT[:, k, :], in0=tp,
                scalar1=g_sb[:, k, :], scalar2=b_sb[:, k, :],
                op0=mybir.AluOpType.mult, op1=mybir.AluOpType.add,
            )

    # =====================================================================
    # Phase 1: LN1 + transpose + QKV per chunk
    # =====================================================================
    def ln_qkv_chunk(c):
        hT = hT_pool.tile([ET, NET, TC], mdt, name="hT")
        x_ts = []
        h_ts = []
        for s in range(NSUB):
            r0 = c * TC + s * TS
            x_t = xt_pool.tile([128, E], f32, name="x_t")
            # alternate DMA queues so the DGE configs run in parallel
            eng = nc.scalar if s % 2 == 0 else nc.sync
            eng.dma_start(out=x_t, in_=xr[r0:r0 + TS, :])
            x_ts.append(x_t)
            h_ts.append(h_pool.tile([128, E], mdt, name="h_t"))
        layer_norm_chunk(x_ts, eps1_sb, h_ts)
        transpose_cast(h_ts, ln_sb["ln1g"], ln_sb["ln1b"], hT)
        for dd in range(NDT):
            for w_sb, dst in ((wq_sb, qT_c[c]), (wk_sb, kT_c[c])):
                ps = ps_mm.tile([128, TC], f32, name="ps_qk", tag="mm")
                for k in range(NET):
                    nc.tensor.matmul(
                        ps, w_sb[:, k, dd * 128:(dd + 1) * 128],
                        hT[:, k, :],
                        start=(k == 0), stop=(k == NET - 1),
                    )
                nc.scalar.copy(dst[:, dd, :], ps)
        for s in range(NSUB):
            ps = ps_mm.tile([128, DSL], f32, name="ps_v", tag="mm")
            for k in range(NET):
                nc.tensor.matmul(
                    ps, hT[:, k, s * TS:(s + 1) * TS], wv_sb[:, k, :],
                    start=(k == 0), stop=(k == NET - 1),
                )
            nc.scalar.copy(
                vt_c[c][:, s, :, 0:HS],
                ps.rearrange("p (h d) -> p h d", h=HPC),
            )
            nc.gpsimd.memset(vt_c[c][:, s, :, HS:HS + 1], 1.0)

    # =====================================================================
    # Phase 2: attention + proj partials
    # =====================================================================
    def attention_chunk(c):
        nkt = (c + 1) * NSUB
        attTs = []
        for pr in range(NDT):  # head pair = d-tile
            av_ps = [ps_av.tile([128, TC], f32, name="avp") for _ in range(2)]

            def do_scores(i):
                m = i - c * NSUB
                w0 = m * TS if m > 0 else 0  # suffix start (diagonal tiles)
                pts = []
                for hh in range(2):
                    h0 = hh * HS
                    sc = ps_sc.tile([TS, TC], f32, name="sc")
                    nc.tensor.matmul(
                        sc[:, w0:TC],
                        kT_c[i // NSUB][h0:h0 + HS, pr,
                                        (i % NSUB) * TS:(i % NSUB + 1) * TS],
                        qT_c[c][h0:h0 + HS, pr, w0:TC],
                        start=True, stop=True,
                    )
                    pt = pt_pool.tile([TS, TC], mdt, name="pt")
                    nc.scalar.activation(
                        out=pt[:, w0:TC], in_=sc[:, w0:TC], func=AF.Exp,
                        scale=SCALE,
                    )
                    if m >= 0:
                        # diagonal block: zero t_k > t_q inside [m*TS,(m+1)*TS)
                        nc.vector.tensor_mul(
                            pt[:, m * TS:(m + 1) * TS],
                            pt[:, m * TS:(m + 1) * TS], mask_sb,
                        )
                    pts.append((pt, w0))
                return pts

            def do_avs(i, pts):
                for hh in range(2):
                    pt, w0 = pts[hh]
                    nc.tensor.matmul(
                        av_ps[hh][0:HS + 1, w0:TC],
                        vt_c[i // NSUB][:, i % NSUB, pr * 2 + hh, :],
                        pt[:, w0:TC],
                        start=(i == 0), stop=(i == nkt - 1),
                        skip_group_check=True,
                    )

            # stagger: emit scores+exp for a group of 4 t_k tiles, then the
            # av matmuls of the previous group, so PE never waits on exp.
            G = 4
            prev = []
            for g0 in range(0, nkt, G):
                cur = [(i, do_scores(i)) for i in range(g0, min(g0 + G, nkt))]
                for i, pts in prev:
                    do_avs(i, pts)
                prev = cur
            for i, pts in prev:
                do_avs(i, pts)

            # --- denominators, fully on-chip: reciprocal of the PSUM ones-row
            # then a K=1 ones matmul broadcasts it into partitions 64:128 of
            # the same PSUM bank; one DVE multiply normalizes.
            attT = attT_pool.tile([128, TC], mdt, name="attT")
            for hh in range(2):
                av_sb = avs_pool.tile([HS + 1, TC], f32, name="av_sb")
                nc.vector.tensor_copy(av_sb, av_ps[hh][0:HS + 1, :])
                recb = rec_pool.tile([1, TC], mdt, name="recb")
                with nc.allow_low_precision(reason="bf16 softmax denominators"):
                    nc.vector.reciprocal(out=recb, in_=av_sb[HS:HS + 1, :])
                nc.tensor.matmul(
                    av_ps[hh][HS:HS + HS, :], ones64b, recb,
                    start=True, stop=True, skip_group_check=True,
                )
                nc.vector.tensor_mul(
                    attT[hh * HS:(hh + 1) * HS, :], av_sb[0:HS, :],
                    av_ps[hh][HS:HS + HS, :],
                )
            attTs.append(attT)
        return attTs

    def proj_chunk(c, attTs):
        # ar_in layout: region r = c%2 rows [r*2TC,(r+1)*2TC): rank-0 chunk
        # first then rank-1 chunk, so each ReduceScatter input is contiguous.
        for s in range(NSUB):
            r0 = c * TC + s * TS
            w0 = (c % 2) * (2 * TC) + (c // 2) * TC + s * TS
            xb_t = xb_pool.tile([128, E], f32, name="xb_t")
            nc.gpsimd.dma_start(out=xb_t, in_=xb[r0:r0 + TS, :])
            part = part_pool.tile([128, E], mdt, name="part")
            for n in range(2):
                ps = ps_mm.tile([128, TC], f32, name="ps_pr", tag="mm")
                for dd in range(NDT):
                    nc.tensor.matmul(
                        ps, attTs[dd][:, s * TS:(s + 1) * TS],
                        wp_sb[:, dd, n * TC:(n + 1) * TC],
                        start=(dd == 0), stop=(dd == NDT - 1),
                    )
                nc.vector.tensor_add(
                    part[:, n * TC:(n + 1) * TC], ps, xb_t[:, n * TC:(n + 1) * TC]
                )
            nc.gpsimd.dma_start(out=ar_in[w0:w0 + TS, :], in_=part)

    # =====================================================================
    # Phase 3: LN2 + FFN + residual for one reduced region (one chunk/core)
    # =====================================================================
    def ffn_region(r):
        h2T = hT_pool.tile([ET, NET, TC], mdt, name="h2T")
        x2_ts = []
        h2_ts = []
        for s in range(NSUB):
            x2_t = x2_pool.tile([128, E], mdt, name="x2_t")
            nc.gpsimd.dma_start(out=x2_t, in_=rs_out[r][s * TS:(s + 1) * TS, :])
            x2_ts.append(x2_t)
            h2_ts.append(h_pool.tile([128, E], mdt, name="h2_t", tag="h_t"))
        layer_norm_chunk(x2_ts, eps2_sb, h2_ts)
        transpose_cast(h2_ts, ln_sb["ln2g"], ln_sb["ln2b"], h2T)
        f1 = f1_pool.tile([FFN + 1, TC], mdt, name="f1")
        nc.vector.memset(f1, 1.0)  # row FFN stays 1.0 (b2 matmul row)
        ps_f = ps_mm.tile([FFN, TC], f32, name="ps_f", tag="mm")
        for k in range(NET):
            nc.tensor.matmul(
                ps_f, w1_sb[:, k, :], h2T[:, k, :],
                start=(k == 0), stop=(k == NET - 1),
            )
        nc.scalar.activation(
            out=f1[0:FFN, :], in_=ps_f, func=AF.Relu, bias=b1_sb, scale=1.0
        )
        for s in range(NSUB):
            o_t = out_pool.tile([128, E], f32, name="o_t")
            for n in range(2):
                ps = ps_mm.tile([128, TC], f32, name="ps_o", tag="mm")
                nc.tensor.matmul(
                    ps, f1[:, s * TS:(s + 1) * TS],
                    w2_sb[:, n * TC:(n + 1) * TC],
                    start=True, stop=True,
                )
                nc.vector.tensor_add(
                    o_t[:, n * TC:(n + 1) * TC], ps,
                    x2_ts[s][:, n * TC:(n + 1) * TC],
                )
            r0 = r * TC + s * TS
            nc.gpsimd.dma_start(out=out[r0:r0 + TS, :], in_=o_t)

    # ---- schedule: pull att(0)/att(2) into the QKV phase so the Act
    # engine's exp load is spread across the whole timeline ----
    ln_qkv_chunk(0)
    ln_qkv_chunk(1)
    ln_qkv_chunk(2)
    attTs0 = attention_chunk(0)
    ln_qkv_chunk(3)
    attTs2 = attention_chunk(2)
    proj_chunk(0, attTs0)
    proj_chunk(2, attTs2)
    nc.gpsimd.collective_compute(
        "ReduceScatter", mybir.AluOpType.add, replica_groups=PAIRS,
        ins=[ar_in[0:2 * TC, :]], outs=[rs_out[0]],
    )
    proj_chunk(1, attention_chunk(1))
    ffn_region(0)            # RS_A long since done; overlaps chunk-3 attention
    proj_chunk(3, attention_chunk(3))
    nc.gpsimd.collective_compute(
        "ReduceScatter", mybir.AluOpType.add, replica_groups=PAIRS,
        ins=[ar_in[2 * TC:4 * TC, :]], outs=[rs_out[1]],
    )
    ffn_region(1)


# =========================================================================
# Host side
# =========================================================================
_NC_CACHE = {}
RUN_KWARGS = {}      # test harness may set {"trace": True} for profiling
LAST_RESULT = None   # BassKernelResults of the most recent run


def kernel(x, wq, wk, wv, w_proj, b_proj, w1, b1, w2, b2, ln1_g, ln1_b, ln2_g,
           ln2_b):
    mode = MM_MODE
    np_mdt = _np_mdt(mode)
    if mode not in _NC_CACHE:
        _NC_CACHE[mode] = build(mode)
    nc = _NC_CACHE[mode]

    x = np.asarray(x, np.float32)
    # lower-triangle [128,128] block mask: valid (1.0) iff t_k(p) <= t_q(f)
    mask = np.tril(np.ones((TS, TS), np.float32)).T.astype(np_mdt)
    identity = np.eye(TS, dtype=np.float32).astype(np_mdt)
    w2e = np.concatenate([np.asarray(w2, np.float32),
                          np.asarray(b2, np.float32)[None, :]], axis=0)
    in_maps = []
    for core in range(NCORE):
        b, g = core // 2, core % 2
        sl = slice(g * DSL, (g + 1) * DSL)
        in_maps.append({
            "xr": 0.5 * x[b],
            "xb": 0.5 * x[b] + 0.5 * np.asarray(b_proj, np.float32)[None, :],
            "wq": np.asarray(wq, np.float32)[:, sl].astype(np_mdt),
            "wk": np.asarray(wk, np.float32)[:, sl].astype(np_mdt),
            "wv": np.asarray(wv, np.float32)[:, sl].astype(np_mdt),
            "wp": np.asarray(w_proj, np.float32)[sl, :].astype(np_mdt),
            "w1": np.asarray(w1, np.float32).astype(np_mdt),
            "w2e": w2e.astype(np_mdt),
            "b1": np.asarray(b1, np.float32)[:, None],
            "ln1g": np.asarray(ln1_g, np.float32)[:, None],
            "ln1b": np.asarray(ln1_b, np.float32)[:, None],
            "ln2g": np.asarray(ln2_g, np.float32)[:, None],
            "ln2b": np.asarray(ln2_b, np.float32)[:, None],
            "mask": mask,
            "ident": identity,
        })
    global LAST_RESULT
    res = run_bass_kernel_spmd(nc, in_maps, list(range(NCORE)), **RUN_KWARGS)
    LAST_RESULT = res
    outp = np.empty((B, T, E), np.float32)
    for core in range(NCORE):
        b, g = core // 2, core % 2
        outp[b, g * (T // 2):(g + 1) * (T // 2), :] = res.results[core]["out"]
    return outp


# revision 29
# speedup vs baseline: 1.1933x; 1.1933x over previous
"""Trainium2 Bass kernel: pre-LN transformer block (B=4, T=2048, E=1024, H=16, FFN=100).

Sharding (8 NeuronCores): core 2b+g handles batch b, head-group g (8 of 16 heads,
i.e. a 512-wide slice of the QKV output dim / proj input dim).  Both cores of a
pair compute attention + proj partials for all 2048 tokens of their batch; two
per-pair ReduceScatters (bf16, Shared outputs) combine the partials and hand
each core half the tokens, on which it runs LN2 + FFN and writes its
[1024, 1024] output shard.

SPMD notes: all 8 cores run one program; per-core behavior differs only via
input data.  The residual is fed as x/2 on both pair members (summed back to x
by the reduce); LN1 uses eps/4 so layernorm(x/2, eps/4) == layernorm(x, eps)
exactly.  b_proj/2 is folded host-side into the proj residual copy of x (xb),
and b2 is folded into the FFN second matmul as an extra input row.

Attention layout: scores are computed transposed, S^T[t_k, t_q] = k^T.T @ q^T,
with q^T/k^T in [head_dim, token] layout (from PE-transposed LN output, all in
bf16 so transposes run at 1 cycle/row).  Softmax runs without max subtraction
(logits are ~N(0, 0.25), safe in fp32): exp on ScalarE straight out of PSUM
with the 1/sqrt(E) scale folded in.  Causality is exploited at 128-column
granularity: for the diagonal t_k tile at offset m only the suffix columns
[m*128:) are computed/exp'd, and a single [128,128] lower-triangle mask fixes
the diagonal block.  The denominator comes from an extra ones-column appended
to V; its reciprocal row is broadcast across partitions 64:128 of the same AV
PSUM bank via a K=1 ones matmul (fully on-chip - no DRAM bounce), and one DVE
multiply produces the normalized attention output.
"""

from contextlib import ExitStack

import numpy as np
import ml_dtypes

import concourse.bass as bass
import concourse.mybir as mybir
import concourse.tile as tile
from concourse.bass_utils import run_bass_kernel_spmd
from concourse.vector_clock import ScopedClock


class SplitDrainTC(tile.TileContext):
    """Works around a walrus codegen limit: an SP CTRL instruction may carry
    only one sync wait, so the kernel-tail drain's waits are split onto
    preceding single-wait nops."""

    def _drain_and_barrier(self, tick_clock, wait_clock):
        probe = self.nc.sync.nop(nofuse=True)
        wait_clock.add_sem_waits(
            probe.ins, ScopedClock({None: tick_clock.global_clock})
        )
        si = probe.ins.sync_info
        waits = list(si.on_wait) if si is not None else []
        if len(waits) > 1:
            si.on_wait = [waits[0]]
            for w in waits[1:]:
                n2 = self.nc.sync.nop(nofuse=True)
                n2.ins.sync_info = mybir.SyncInfo(on_wait=[w], on_update=[])
        self.nc.sync.drain()
        self.nc.all_engine_barrier()
        popped = self.nc._tile_sem_poison_stack.pop()
        assert popped is self._sem_poison
        self.nc.clear_and_free_semaphores(list(self.sems.allocated().values()))
        self.nc.all_engine_barrier()

B, T, E, H, HS, FFN = 4, 2048, 1024, 16, 64, 100
EPS = 1e-5
NCORE = 8
TC = 512            # token chunk
NTC = T // TC       # 4
TS = 128            # token subtile
NSUB = TC // TS     # 4
ET = 128            # embed tile
NET = E // ET       # 8
DSL = E // 2        # per-core qkv output slice (8 heads * 64)
NDT = DSL // 128    # 4 d-tiles (2 heads each)
HPC = H // 2        # 8 heads per core
SCALE = float(E) ** -0.5
PAIRS = [[0, 1], [2, 3], [4, 5], [6, 7]]

MM_MODE = "bf16"    # "bf16" | "f32"
AF = mybir.ActivationFunctionType


def _mdt(mode):
    return mybir.dt.bfloat16 if mode == "bf16" else mybir.dt.float32


def _np_mdt(mode):
    return ml_dtypes.bfloat16 if mode == "bf16" else np.float32


def build(mode=MM_MODE):
    f32 = mybir.dt.float32
    mdt = _mdt(mode)

    nc = bass.Bass(num_devices=NCORE)

    io = {}

    def param(name, shape, dtype):
        io[name] = nc.declare_dram_parameter(name, shape, dtype, isOutput=False)

    param("xr", [T, E], f32)           # x/2 (LN1 input)
    param("xb", [T, E], f32)           # x/2 + b_proj/2 (proj residual)
    param("wq", [E, DSL], mdt)
    param("wk", [E, DSL], mdt)
    param("wv", [E, DSL], mdt)
    param("wp", [DSL, E], mdt)
    param("w1", [E, FFN], mdt)
    param("w2e", [FFN + 1, E], mdt)    # w2 with b2 as the extra last row
    param("b1", [FFN, 1], f32)
    param("ln1g", [E, 1], f32)
    param("ln1b", [E, 1], f32)
    param("ln2g", [E, 1], f32)
    param("ln2b", [E, 1], f32)
    param("mask", [TS, TS], mdt)       # lower-triangle (t_k <= t_q) block mask
    param("ident", [TS, TS], mdt)
    io["out"] = nc.declare_dram_parameter("out", [T // 2, E], f32, isOutput=True)

    with SplitDrainTC(nc) as tc:
        with ExitStack() as ctx:
            _build_tile(ctx, tc, nc, mode, mdt, f32, io)
    _split_waits(nc)
    return nc


def _split_waits(nc, maxw=1):
    """walrus codegen accepts a limited number of sync waits per instruction;
    move the excess onto same-engine NoOps inserted just before."""
    import bass_rust
    n = 0
    for f in nc.m.functions:
        for b in f.blocks:
            new = []
            for inst in b.instructions:
                si = inst.sync_info
                # fixed-length ISA instructions can't carry waits at all
                cap = 0 if isinstance(inst, bass_rust.InstISA) else maxw
                if si is not None and len(si.on_wait) > cap:
                    waits = list(si.on_wait)
                    keep = waits[-cap:] if cap else []
                    excess = waits[:-cap] if cap else waits
                    for w in excess:
                        nop = mybir.InstNoOp(
                            name=f"{inst.name}-wsplit{n}", engine=inst.engine
                        )
                        nop.bass_nofuse = True
                        n += 1
                        nop.sync_info = mybir.SyncInfo(
                            on_wait=[w], on_update=[]
                        )
                        new.append(nop)
                    si.on_wait = keep
                new.append(inst)
            if n:
                b.instructions = new


def _build_tile(ctx, tc, nc, mode, mdt, f32, io):
    xr, xb, out = io["xr"], io["xb"], io["out"]

    def pool(name, bufs, space="SBUF"):
        return ctx.enter_context(tc.tile_pool(name=name, bufs=bufs, space=space))

    # ---- internal DRAM ----
    dram = pool("dram", 1, space="DRAM")
    ar_in = dram.tile([T, E], mdt, name="ar_in")
    rs_out = [
        dram.tile([TC, E], mdt, name="rs0_out"),
        dram.tile([TC, E], mdt, name="rs1_out"),
    ]

    # ---- persistent SBUF: weights & constants (DMAs on the idle Pool
    # sequencer; its DGE config time is ~25ns vs SP's ~565ns) ----
    wpool = pool("weights", 1)
    wq_sb = wpool.tile([ET, NET, DSL], mdt, name="wq_sb")
    wk_sb = wpool.tile([ET, NET, DSL], mdt, name="wk_sb")
    wv_sb = wpool.tile([ET, NET, DSL], mdt, name="wv_sb")
    # big weights go on the (otherwise idle) SP queue so the gpsimd queue can
    # start streaming x tiles immediately
    nc.sync.dma_start(out=wq_sb, in_=io["wq"].rearrange("(k p) d -> p k d", p=ET))
    nc.sync.dma_start(out=wk_sb, in_=io["wk"].rearrange("(k p) d -> p k d", p=ET))
    nc.sync.dma_start(out=wv_sb, in_=io["wv"].rearrange("(k p) d -> p k d", p=ET))
    wp_sb = wpool.tile([128, NDT, E], mdt, name="wp_sb")
    nc.sync.dma_start(out=wp_sb, in_=io["wp"].rearrange("(k p) d -> p k d", p=128))
    ones64b = wpool.tile([1, HS], mdt, name="ones64b")
    nc.vector.memset(ones64b, 1.0)
    w1_sb = wpool.tile([ET, NET, FFN], mdt, name="w1_sb")
    nc.sync.dma_start(out=w1_sb, in_=io["w1"].rearrange("(k p) d -> p k d", p=ET))
    w2_sb = wpool.tile([FFN + 1, E], mdt, name="w2_sb")
    nc.sync.dma_start(out=w2_sb, in_=io["w2e"][:])
    b1_sb = wpool.tile([FFN, 1], f32, name="b1_sb")
    nc.sync.dma_start(out=b1_sb, in_=io["b1"][:])
    ln_sb = {}
    for nm in ("ln1g", "ln1b", "ln2g", "ln2b"):
        ln_sb[nm] = wpool.tile([ET, NET, 1], f32, name=nm + "_sb")
        nc.gpsimd.dma_start(
            out=ln_sb[nm], in_=io[nm].rearrange("(k p) o -> p k o", p=ET)
        )
    mask_sb = wpool.tile([TS, TS], mdt, name="mask_sb")
    nc.gpsimd.dma_start(out=mask_sb, in_=io["mask"][:])
    id_sb = wpool.tile([TS, TS], mdt, name="id_sb")
    nc.gpsimd.dma_start(out=id_sb, in_=io["ident"][:])
    eps1_sb = wpool.tile([128, 1], f32, name="eps1_sb")
    nc.vector.memset(eps1_sb, EPS / 4.0)  # LN1 runs on x/2
    eps2_sb = wpool.tile([128, 1], f32, name="eps2_sb")
    nc.vector.memset(eps2_sb, EPS)

    # ---- persistent SBUF: per-chunk K^T, V(+ones), Q^T ----
    kv = pool("kv", 1)
    kT_c = [kv.tile([128, NDT, TC], mdt, name=f"kT{c}") for c in range(NTC)]
    vt_c = [kv.tile([128, NSUB, HPC, HS + 1], mdt, name=f"vt{c}")
            for c in range(NTC)]
    qT_c = [kv.tile([128, NDT, TC], mdt, name=f"qT{c}") for c in range(NTC)]

    # ---- working pools ----
    xt_pool = pool("xt", 4)
    h_pool = pool("h", 5)
    mv_pool = pool("mv", 3)
    hT_pool = pool("hT", 2)
    pt_pool = pool("pt", 6)
    avs_pool = pool("avs", 4)
    rec_pool = pool("rec", 4)
    attT_pool = pool("attT", 8)   # att(0) and att(2) outputs both live
    xb_pool = pool("xbp", 2)
    part_pool = pool("part", 3)
    x2_pool = pool("x2", 4)
    f1_pool = pool("f1", 2)
    out_pool = pool("outp", 2)
    ps_mm = pool("ps_mm", 2, space="PSUM")
    ps_sc = pool("ps_sc", 4, space="PSUM")
    ps_av = pool("ps_av", 2, space="PSUM")

    def layer_norm_chunk(x_ts, eps_tile, out_ts):
        """out_ts[s] (bf16) = (x_ts[s] - mean) * rsqrt(var + eps), with the
        4 subtiles' stats batched so Sqrt costs one Act instruction (one
        activation-table region instead of four)."""
        n = len(x_ts)
        mv = mv_pool.tile([128, n, 2], f32, name="mv")
        for s, x_t in enumerate(x_ts):
            stats = mv_pool.tile(
                [128, 2, nc.vector.BN_STATS_DIM], f32, name="stats"
            )
            xg = x_t.rearrange("p (u q) -> p u q", u=2)
            for u in range(2):
                nc.vector.bn_stats(out=stats[:, u, :], in_=xg[:, u, :])
            nc.vector.bn_aggr(out=mv[:, s, :], in_=stats)
        rstd = mv_pool.tile([128, n], f32, name="rstd")
        nc.scalar.activation(
            out=rstd, in_=mv[:, :, 1], func=AF.Sqrt, bias=eps_tile, scale=1.0
        )
        nc.vector.reciprocal(out=rstd, in_=rstd)
        for s, x_t in enumerate(x_ts):
            nc.vector.tensor_scalar(
                out=out_ts[s], in0=x_t, scalar1=mv[:, s, 0:1],
                scalar2=rstd[:, s:s + 1],
                op0=mybir.AluOpType.subtract, op1=mybir.AluOpType.mult,
            )

    def transpose_cast(h_ts, g_sb, b_sb, hT):
        """PE-transpose 4 subtiles of h [128, E] (bf16) into hT[:, k, :],
        batching the 4 128x128 transposes of one k-tile into one PSUM tile so
        the layernorm scale/bias fold costs one DVE op per [128, 512]."""
        for k in range(NET):
            tp = ps_mm.tile([TS, TC], mdt, name="tp", tag="mm")
            for s in range(NSUB):
                nc.tensor.transpose(
                    tp[:, s * TS:(s + 1) * TS],
                    h_ts[s][:, k * ET:(k + 1) * ET], id_sb,
                )
            nc.vector.tensor_scalar(
                out=hT[:, k, :], in0=tp,
                scalar1=g_sb[:, k, :], scalar2=b_sb[:, k, :],
                op0=mybir.AluOpType.mult, op1=mybir.AluOpType.add,
            )

    # =====================================================================
    # Phase 1: LN1 + transpose + QKV per chunk
    # =====================================================================
    def ln_qkv_chunk(c):
        hT = hT_pool.tile([ET, NET, TC], mdt, name="hT")
        x_ts = []
        h_ts = []
        for s in range(NSUB):
            r0 = c * TC + s * TS
            x_t = xt_pool.tile([128, E], f32, name="x_t")
            # alternate DMA queues so the DGE configs run in parallel
            eng = nc.scalar if s % 2 == 0 else nc.sync
            eng.dma_start(out=x_t, in_=xr[r0:r0 + TS, :])
            x_ts.append(x_t)
            h_ts.append(h_pool.tile([128, E], mdt, name="h_t"))
        layer_norm_chunk(x_ts, eps1_sb, h_ts)
        transpose_cast(h_ts, ln_sb["ln1g"], ln_sb["ln1b"], hT)
        for dd in range(NDT):
            for w_sb, dst in ((wq_sb, qT_c[c]), (wk_sb, kT_c[c])):
                ps = ps_mm.tile([128, TC], f32, name="ps_qk", tag="mm")
                for k in range(NET):
                    nc.tensor.matmul(
                        ps, w_sb[:, k, dd * 128:(dd + 1) * 128],
                        hT[:, k, :],
                        start=(k == 0), stop=(k == NET - 1),
                    )
                nc.scalar.copy(dst[:, dd, :], ps)
        for s in range(NSUB):
            ps = ps_mm.tile([128, DSL], f32, name="ps_v", tag="mm")
            for k in range(NET):
                nc.tensor.matmul(
                    ps, hT[:, k, s * TS:(s + 1) * TS], wv_sb[:, k, :],
                    start=(k == 0), stop=(k == NET - 1),
                )
            nc.scalar.copy(
                vt_c[c][:, s, :, 0:HS],
                ps.rearrange("p (h d) -> p h d", h=HPC),
            )
            nc.gpsimd.memset(vt_c[c][:, s, :, HS:HS + 1], 1.0)

    # =====================================================================
    # Phase 2: attention + proj partials
    # =====================================================================
    def attention_chunk(c):
        nkt = (c + 1) * NSUB
        attTs = []
        for pr in range(NDT):  # head pair = d-tile
            av_ps = [ps_av.tile([128, TC], f32, name="avp") for _ in range(2)]

            def do_scores(i):
                m = i - c * NSUB
                w0 = m * TS if m > 0 else 0  # suffix start (diagonal tiles)
                pts = []
                for hh in range(2):
                    h0 = hh * HS
                    sc = ps_sc.tile([TS, TC], f32, name="sc")
                    nc.tensor.matmul(
                        sc[:, w0:TC],
                        kT_c[i // NSUB][h0:h0 + HS, pr,
                                        (i % NSUB) * TS:(i % NSUB + 1) * TS],
                        qT_c[c][h0:h0 + HS, pr, w0:TC],
                        start=True, stop=True,
                    )
                    pt = pt_pool.tile([TS, TC], mdt, name="pt")
                    nc.scalar.activation(
                        out=pt[:, w0:TC], in_=sc[:, w0:TC], func=AF.Exp,
                        scale=SCALE,
                    )
                    if m >= 0:
                        # diagonal block: zero t_k > t_q inside [m*TS,(m+1)*TS)
                        nc.vector.tensor_mul(
                            pt[:, m * TS:(m + 1) * TS],
                            pt[:, m * TS:(m + 1) * TS], mask_sb,
                        )
                    pts.append((pt, w0))
                return pts

            def do_avs(i, pts):
                for hh in range(2):
                    pt, w0 = pts[hh]
                    nc.tensor.matmul(
                        av_ps[hh][0:HS + 1, w0:TC],
                        vt_c[i // NSUB][:, i % NSUB, pr * 2 + hh, :],
                        pt[:, w0:TC],
                        start=(i == 0), stop=(i == nkt - 1),
                        skip_group_check=True,
                    )

            # stagger: emit scores+exp for a group of 4 t_k tiles, then the
            # av matmuls of the previous group, so PE never waits on exp.
            G = 4
            prev = []
            for g0 in range(0, nkt, G):
                cur = [(i, do_scores(i)) for i in range(g0, min(g0 + G, nkt))]
                for i, pts in prev:
                    do_avs(i, pts)
                prev = cur
            for i, pts in prev:
                do_avs(i, pts)

            # --- denominators, fully on-chip: reciprocal of the PSUM ones-row
            # then a K=1 ones matmul broadcasts it into partitions 64:128 of
            # the same PSUM bank; one DVE multiply normalizes.
            attT = attT_pool.tile([128, TC], mdt, name="attT")
            for hh in range(2):
                av_sb = avs_pool.tile([HS + 1, TC], f32, name="av_sb")
                nc.vector.tensor_copy(av_sb, av_ps[hh][0:HS + 1, :])
                recb = rec_pool.tile([1, TC], mdt, name="recb")
                with nc.allow_low_precision(reason="bf16 softmax denominators"):
                    nc.vector.reciprocal(out=recb, in_=av_sb[HS:HS + 1, :])
                nc.tensor.matmul(
                    av_ps[hh][HS:HS + HS, :], ones64b, recb,
                    start=True, stop=True, skip_group_check=True,
                )
                nc.vector.tensor_mul(
                    attT[hh * HS:(hh + 1) * HS, :], av_sb[0:HS, :],
                    av_ps[hh][HS:HS + HS, :],
                )
            attTs.append(attT)
        return attTs

    def proj_chunk(c, attTs):
        # ar_in layout: region r = c%2 rows [r*2TC,(r+1)*2TC): rank-0 chunk
        # first then rank-1 chunk, so each ReduceScatter input is contiguous.
        for s in range(NSUB):
            r0 = c * TC + s * TS
            w0 = (c % 2) * (2 * TC) + (c // 2) * TC + s * TS
            xb_t = xb_pool.tile([128, E], f32, name="xb_t")
            nc.gpsimd.dma_start(out=xb_t, in_=xb[r0:r0 + TS, :])
            part = part_pool.tile([128, E], mdt, name="part")
            for n in range(2):
                ps = ps_mm.tile([128, TC], f32, name="ps_pr", tag="mm")
                for dd in range(NDT):
                    nc.tensor.matmul(
                        ps, attTs[dd][:, s * TS:(s + 1) * TS],
                        wp_sb[:, dd, n * TC:(n + 1) * TC],
                        start=(dd == 0), stop=(dd == NDT - 1),
                    )
                nc.vector.tensor_add(
                    part[:, n * TC:(n + 1) * TC], ps, xb_t[:, n * TC:(n + 1) * TC]
                )
            nc.gpsimd.dma_start(out=ar_in[w0:w0 + TS, :], in_=part)

    # =====================================================================
    # Phase 3: LN2 + FFN + residual for one reduced region (one chunk/core)
    # =====================================================================
    def ffn_region(r):
        h2T = hT_pool.tile([ET, NET, TC], mdt, name="h2T")
        x2_ts = []
        h2_ts = []
        for s in range(NSUB):
            x2_t = x2_pool.tile([128, E], mdt, name="x2_t")
            nc.gpsimd.dma_start(out=x2_t, in_=rs_out[r][s * TS:(s + 1) * TS, :])
            x2_ts.append(x2_t)
            h2_ts.append(h_pool.tile([128, E], mdt, name="h2_t", tag="h_t"))
        layer_norm_chunk(x2_ts, eps2_sb, h2_ts)
        transpose_cast(h2_ts, ln_sb["ln2g"], ln_sb["ln2b"], h2T)
        f1 = f1_pool.tile([FFN + 1, TC], mdt, name="f1")
        nc.vector.memset(f1, 1.0)  # row FFN stays 1.0 (b2 matmul row)
        ps_f = ps_mm.tile([FFN, TC], f32, name="ps_f", tag="mm")
        for k in range(NET):
            nc.tensor.matmul(
                ps_f, w1_sb[:, k, :], h2T[:, k, :],
                start=(k == 0), stop=(k == NET - 1),
            )
        nc.scalar.activation(
            out=f1[0:FFN, :], in_=ps_f, func=AF.Relu, bias=b1_sb, scale=1.0
        )
        for s in range(NSUB):
            o_t = out_pool.tile([128, E], f32, name="o_t")
            for n in range(2):
                ps = ps_mm.tile([128, TC], f32, name="ps_o", tag="mm")
                nc.tensor.matmul(
                    ps, f1[:, s * TS:(s + 1) * TS],
                    w2_sb[:, n * TC:(n + 1) * TC],
                    start=True, stop=True,
                )
                nc.vector.tensor_add(
                    o_t[:, n * TC:(n + 1) * TC], ps,
                    x2_ts[s][:, n * TC:(n + 1) * TC],
                )
            r0 = r * TC + s * TS
            nc.gpsimd.dma_start(out=out[r0:r0 + TS, :], in_=o_t)

    # ---- schedule: pull att(0)/att(2) into the QKV phase so the Act
    # engine's exp load is spread across the whole timeline ----
    ln_qkv_chunk(0)
    ln_qkv_chunk(1)
    ln_qkv_chunk(2)
    attTs0 = attention_chunk(0)
    ln_qkv_chunk(3)
    attTs2 = attention_chunk(2)
    proj_chunk(0, attTs0)
    proj_chunk(2, attTs2)
    nc.gpsimd.collective_compute(
        "ReduceScatter", mybir.AluOpType.add, replica_groups=PAIRS,
        ins=[ar_in[0:2 * TC, :]], outs=[rs_out[0]],
    )
    proj_chunk(1, attention_chunk(1))
    ffn_region(0)            # RS_A long since done; overlaps chunk-3 attention
    proj_chunk(3, attention_chunk(3))
    nc.gpsimd.collective_compute(
        "ReduceScatter", mybir.AluOpType.add, replica_groups=PAIRS,
        ins=[ar_in[2 * TC:4 * TC, :]], outs=[rs_out[1]],
    )
    ffn_region(1)


# =========================================================================
# Host side
# =========================================================================
_NC_CACHE = {}
RUN_KWARGS = {}      # test harness may set {"trace": True} for profiling
LAST_RESULT = None   # BassKernelResults of the most recent run


def kernel(x, wq, wk, wv, w_proj, b_proj, w1, b1, w2, b2, ln1_g, ln1_b, ln2_g,
           ln2_b):
    mode = MM_MODE
    np_mdt = _np_mdt(mode)
    if mode not in _NC_CACHE:
        _NC_CACHE[mode] = build(mode)
    nc = _NC_CACHE[mode]

    x = np.asarray(x, np.float32)
    # lower-triangle [128,128] block mask: valid (1.0) iff t_k(p) <= t_q(f)
    mask = np.tril(np.ones((TS, TS), np.float32)).T.astype(np_mdt)
    identity = np.eye(TS, dtype=np.float32).astype(np_mdt)
    w2e = np.concatenate([np.asarray(w2, np.float32),
                          np.asarray(b2, np.float32)[None, :]], axis=0)
    in_maps = []
    for core in range(NCORE):
        b, g = core // 2, core % 2
        sl = slice(g * DSL, (g + 1) * DSL)
        in_maps.append({
            "xr": 0.5 * x[b],
            "xb": 0.5 * x[b] + 0.5 * np.asarray(b_proj, np.float32)[None, :],
            "wq": np.asarray(wq, np.float32)[:, sl].astype(np_mdt),
            "wk": np.asarray(wk, np.float32)[:, sl].astype(np_mdt),
            "wv": np.asarray(wv, np.float32)[:, sl].astype(np_mdt),
            "wp": np.asarray(w_proj, np.float32)[sl, :].astype(np_mdt),
            "w1": np.asarray(w1, np.float32).astype(np_mdt),
            "w2e": w2e.astype(np_mdt),
            "b1": np.asarray(b1, np.float32)[:, None],
            "ln1g": np.asarray(ln1_g, np.float32)[:, None],
            "ln1b": np.asarray(ln1_b, np.float32)[:, None],
            "ln2g": np.asarray(ln2_g, np.float32)[:, None],
            "ln2b": np.asarray(ln2_b, np.float32)[:, None],
            "mask": mask,
            "ident": identity,
        })
    global LAST_RESULT
    res = run_bass_kernel_spmd(nc, in_maps, list(range(NCORE)), **RUN_KWARGS)
    LAST_RESULT = res
    outp = np.empty((B, T, E), np.float32)
    for core in range(NCORE):
        b, g = core // 2, core % 2
        outp[b, g * (T // 2):(g + 1) * (T // 2), :] = res.results[core]["out"]
    return outp


# revision 34
# speedup vs baseline: 1.2306x; 1.0313x over previous
"""Trainium2 Bass kernel: pre-LN transformer block (B=4, T=2048, E=1024, H=16, FFN=100).

Sharding (8 NeuronCores): core 2b+g handles batch b, head-group g (8 of 16 heads,
i.e. a 512-wide slice of the QKV output dim / proj input dim).  Both cores of a
pair compute attention + proj partials for all 2048 tokens of their batch; two
per-pair ReduceScatters (bf16, Shared outputs) combine the partials and hand
each core half the tokens, on which it runs LN2 + FFN and writes its
[1024, 1024] output shard.

SPMD notes: all 8 cores run one program; per-core behavior differs only via
input data.  The residual is fed as x/2 on both pair members (summed back to x
by the reduce); LN1 uses eps/4 so layernorm(x/2, eps/4) == layernorm(x, eps)
exactly.  b_proj/2 is folded host-side into the proj residual copy of x (xb),
and b2 is folded into the FFN second matmul as an extra input row.

Attention layout: scores are computed transposed, S^T[t_k, t_q] = k^T.T @ q^T,
with q^T/k^T in [head_dim, token] layout (from PE-transposed LN output, all in
bf16 so transposes run at 1 cycle/row).  Softmax runs without max subtraction
(logits are ~N(0, 0.25), safe in fp32): exp on ScalarE straight out of PSUM
with the 1/sqrt(E) scale folded in.  Causality is exploited at 128-column
granularity: for the diagonal t_k tile at offset m only the suffix columns
[m*128:) are computed/exp'd, and a single [128,128] lower-triangle mask fixes
the diagonal block.  The denominator comes from an extra ones-column appended
to V; its reciprocal row is broadcast across partitions 64:128 of the same AV
PSUM bank via a K=1 ones matmul (fully on-chip - no DRAM bounce), and one DVE
multiply produces the normalized attention output.
"""

from contextlib import ExitStack

import numpy as np
import ml_dtypes

import concourse.bass as bass
import concourse.mybir as mybir
import concourse.tile as tile
from concourse.bass_utils import run_bass_kernel_spmd
from concourse.vector_clock import ScopedClock


class SplitDrainTC(tile.TileContext):
    """Works around a walrus codegen limit: an SP CTRL instruction may carry
    only one sync wait, so the kernel-tail drain's waits are split onto
    preceding single-wait nops."""

    def _drain_and_barrier(self, tick_clock, wait_clock):
        probe = self.nc.sync.nop(nofuse=True)
        wait_clock.add_sem_waits(
            probe.ins, ScopedClock({None: tick_clock.global_clock})
        )
        si = probe.ins.sync_info
        waits = list(si.on_wait) if si is not None else []
        if len(waits) > 1:
            si.on_wait = [waits[0]]
            for w in waits[1:]:
                n2 = self.nc.sync.nop(nofuse=True)
                n2.ins.sync_info = mybir.SyncInfo(on_wait=[w], on_update=[])
        self.nc.sync.drain()
        self.nc.all_engine_barrier()
        popped = self.nc._tile_sem_poison_stack.pop()
        assert popped is self._sem_poison
        self.nc.clear_and_free_semaphores(list(self.sems.allocated().values()))
        self.nc.all_engine_barrier()

B, T, E, H, HS, FFN = 4, 2048, 1024, 16, 64, 100
EPS = 1e-5
NCORE = 8
TC = 512            # token chunk
NTC = T // TC       # 4
TS = 128            # token subtile
NSUB = TC // TS     # 4
ET = 128            # embed tile
NET = E // ET       # 8
DSL = E // 2        # per-core qkv output slice (8 heads * 64)
NDT = DSL // 128    # 4 d-tiles (2 heads each)
HPC = H // 2        # 8 heads per core
SCALE = float(E) ** -0.5
PAIRS = [[0, 1], [2, 3], [4, 5], [6, 7]]

MM_MODE = "bf16"    # "bf16" | "f32"
AF = mybir.ActivationFunctionType


def _mdt(mode):
    return mybir.dt.bfloat16 if mode == "bf16" else mybir.dt.float32


def _np_mdt(mode):
    return ml_dtypes.bfloat16 if mode == "bf16" else np.float32


def build(mode=MM_MODE):
    f32 = mybir.dt.float32
    mdt = _mdt(mode)

    nc = bass.Bass(num_devices=NCORE)

    io = {}

    def param(name, shape, dtype):
        io[name] = nc.declare_dram_parameter(name, shape, dtype, isOutput=False)

    param("xr", [T, E], f32)           # x/2 (LN1 input)
    param("xb", [T, E], f32)           # x/2 + b_proj/2 (proj residual)
    param("wq", [E, DSL], mdt)
    param("wk", [E, DSL], mdt)
    param("wv", [E, DSL], mdt)
    param("wp", [DSL, E], mdt)
    param("w1", [E, FFN], mdt)
    param("w2e", [FFN + 1, E], mdt)    # w2 with b2 as the extra last row
    param("b1", [FFN, 1], f32)
    param("ln1g", [E, 1], f32)
    param("ln1b", [E, 1], f32)
    param("ln2g", [E, 1], f32)
    param("ln2b", [E, 1], f32)
    param("mask", [TS, TS], mdt)       # lower-triangle (t_k <= t_q) block mask
    param("ident", [TS, TS], mdt)
    io["out"] = nc.declare_dram_parameter("out", [T // 2, E], f32, isOutput=True)

    with SplitDrainTC(nc) as tc:
        with ExitStack() as ctx:
            _build_tile(ctx, tc, nc, mode, mdt, f32, io)
    _split_waits(nc)
    return nc


def _split_waits(nc, maxw=1):
    """walrus codegen accepts a limited number of sync waits per instruction;
    move the excess onto same-engine NoOps inserted just before."""
    import bass_rust
    n = 0
    for f in nc.m.functions:
        for b in f.blocks:
            new = []
            for inst in b.instructions:
                si = inst.sync_info
                # fixed-length ISA instructions can't carry waits at all
                cap = 0 if isinstance(inst, bass_rust.InstISA) else maxw
                if si is not None and len(si.on_wait) > cap:
                    waits = list(si.on_wait)
                    keep = waits[-cap:] if cap else []
                    excess = waits[:-cap] if cap else waits
                    for w in excess:
                        nop = mybir.InstNoOp(
                            name=f"{inst.name}-wsplit{n}", engine=inst.engine
                        )
                        nop.bass_nofuse = True
                        n += 1
                        nop.sync_info = mybir.SyncInfo(
                            on_wait=[w], on_update=[]
                        )
                        new.append(nop)
                    si.on_wait = keep
                new.append(inst)
            if n:
                b.instructions = new


def _build_tile(ctx, tc, nc, mode, mdt, f32, io):
    xr, xb, out = io["xr"], io["xb"], io["out"]

    def pool(name, bufs, space="SBUF"):
        return ctx.enter_context(tc.tile_pool(name=name, bufs=bufs, space=space))

    # ---- internal DRAM ----
    dram = pool("dram", 1, space="DRAM")
    ar_in = dram.tile([T, E], mdt, name="ar_in")
    rs_out = [
        dram.tile([TC, E], mdt, name="rs0_out"),
        dram.tile([TC, E], mdt, name="rs1_out"),
    ]

    # ---- persistent SBUF: weights & constants (DMAs on the idle Pool
    # sequencer; its DGE config time is ~25ns vs SP's ~565ns) ----
    wpool = pool("weights", 1)
    wq_sb = wpool.tile([ET, NET, DSL], mdt, name="wq_sb")
    wk_sb = wpool.tile([ET, NET, DSL], mdt, name="wk_sb")
    wv_sb = wpool.tile([ET, NET, DSL], mdt, name="wv_sb")
    # big weights go on the (otherwise idle) SP queue so the gpsimd queue can
    # start streaming x tiles immediately
    nc.sync.dma_start(out=wq_sb, in_=io["wq"].rearrange("(k p) d -> p k d", p=ET))
    nc.sync.dma_start(out=wk_sb, in_=io["wk"].rearrange("(k p) d -> p k d", p=ET))
    nc.sync.dma_start(out=wv_sb, in_=io["wv"].rearrange("(k p) d -> p k d", p=ET))
    wp_sb = wpool.tile([128, NDT, E], mdt, name="wp_sb")
    nc.sync.dma_start(out=wp_sb, in_=io["wp"].rearrange("(k p) d -> p k d", p=128))
    ones64b = wpool.tile([1, HS], mdt, name="ones64b")
    nc.vector.memset(ones64b, 1.0)
    w1_sb = wpool.tile([ET, NET, FFN], mdt, name="w1_sb")
    nc.sync.dma_start(out=w1_sb, in_=io["w1"].rearrange("(k p) d -> p k d", p=ET))
    w2_sb = wpool.tile([FFN + 1, E], mdt, name="w2_sb")
    nc.sync.dma_start(out=w2_sb, in_=io["w2e"][:])
    b1_sb = wpool.tile([FFN, 1], f32, name="b1_sb")
    nc.sync.dma_start(out=b1_sb, in_=io["b1"][:])
    ln_sb = {}
    for nm in ("ln1g", "ln1b", "ln2g", "ln2b"):
        ln_sb[nm] = wpool.tile([ET, NET, 1], f32, name=nm + "_sb")
        nc.gpsimd.dma_start(
            out=ln_sb[nm], in_=io[nm].rearrange("(k p) o -> p k o", p=ET)
        )
    mask_sb = wpool.tile([TS, TS], mdt, name="mask_sb")
    nc.gpsimd.dma_start(out=mask_sb, in_=io["mask"][:])
    id_sb = wpool.tile([TS, TS], mdt, name="id_sb")
    nc.gpsimd.dma_start(out=id_sb, in_=io["ident"][:])
    eps1_sb = wpool.tile([128, 1], f32, name="eps1_sb")
    nc.vector.memset(eps1_sb, EPS / 4.0)  # LN1 runs on x/2
    eps2_sb = wpool.tile([128, 1], f32, name="eps2_sb")
    nc.vector.memset(eps2_sb, EPS)

    # ---- persistent SBUF: per-chunk K^T, V(+ones), Q^T ----
    kv = pool("kv", 1)
    kT_c = [kv.tile([128, NDT, TC], mdt, name=f"kT{c}") for c in range(NTC)]
    vt_c = [kv.tile([128, NSUB, HPC, HS + 1], mdt, name=f"vt{c}")
            for c in range(NTC)]
    qT_c = [kv.tile([128, NDT, TC], mdt, name=f"qT{c}") for c in range(NTC)]

    # ---- working pools ----
    xt_pool = pool("xt", 4)
    h_pool = pool("h", 5)
    mv_pool = pool("mv", 3)
    hT_pool = pool("hT", 2)
    pt_pool = pool("pt", 6)
    avs_pool = pool("avs", 4)
    rec_pool = pool("rec", 4)
    attT_pool = pool("attT", 8)   # att(0) and att(2) outputs both live
    xb_pool = pool("xbp", 2)
    part_pool = pool("part", 3)
    x2_pool = pool("x2", 4)
    f1_pool = pool("f1", 2)
    out_pool = pool("outp", 2)
    ps_mm = pool("ps_mm", 2, space="PSUM")
    ps_sc = pool("ps_sc", 4, space="PSUM")
    ps_av = pool("ps_av", 2, space="PSUM")

    def layer_norm_chunk(x_ts, eps_tile, out_ts):
        """out_ts[s] (bf16) = (x_ts[s] - mean) * rsqrt(var + eps), with the
        4 subtiles' stats batched so Sqrt costs one Act instruction (one
        activation-table region instead of four)."""
        n = len(x_ts)
        mv = mv_pool.tile([128, n, 2], f32, name="mv")
        for s, x_t in enumerate(x_ts):
            stats = mv_pool.tile(
                [128, 2, nc.vector.BN_STATS_DIM], f32, name="stats"
            )
            xg = x_t.rearrange("p (u q) -> p u q", u=2)
            for u in range(2):
                nc.vector.bn_stats(out=stats[:, u, :], in_=xg[:, u, :])
            nc.vector.bn_aggr(out=mv[:, s, :], in_=stats)
        rstd = mv_pool.tile([128, n], f32, name="rstd")
        nc.scalar.activation(
            out=rstd, in_=mv[:, :, 1], func=AF.Sqrt, bias=eps_tile, scale=1.0
        )
        nc.vector.reciprocal(out=rstd, in_=rstd)
        for s, x_t in enumerate(x_ts):
            nc.vector.tensor_scalar(
                out=out_ts[s], in0=x_t, scalar1=mv[:, s, 0:1],
                scalar2=rstd[:, s:s + 1],
                op0=mybir.AluOpType.subtract, op1=mybir.AluOpType.mult,
            )

    def transpose_cast(h_ts, g_sb, b_sb, hT):
        """PE-transpose 4 subtiles of h [128, E] (bf16) into hT[:, k, :],
        batching the 4 128x128 transposes of one k-tile into one PSUM tile so
        the layernorm scale/bias fold costs one DVE op per [128, 512]."""
        for k in range(NET):
            tp = ps_mm.tile([TS, TC], mdt, name="tp", tag="mm")
            for s in range(NSUB):
                nc.tensor.transpose(
                    tp[:, s * TS:(s + 1) * TS],
                    h_ts[s][:, k * ET:(k + 1) * ET], id_sb,
                )
            nc.vector.tensor_scalar(
                out=hT[:, k, :], in0=tp,
                scalar1=g_sb[:, k, :], scalar2=b_sb[:, k, :],
                op0=mybir.AluOpType.mult, op1=mybir.AluOpType.add,
            )

    # =====================================================================
    # Phase 1: LN1 + transpose + QKV per chunk
    # =====================================================================
    def ln_qkv_chunk(c):
        hT = hT_pool.tile([ET, NET, TC], mdt, name="hT")
        x_ts = []
        h_ts = []
        for s in range(NSUB):
            r0 = c * TC + s * TS
            x_t = xt_pool.tile([128, E], f32, name="x_t")
            nc.gpsimd.dma_start(out=x_t, in_=xr[r0:r0 + TS, :])
            x_ts.append(x_t)
            h_ts.append(h_pool.tile([128, E], mdt, name="h_t"))
        layer_norm_chunk(x_ts, eps1_sb, h_ts)
        transpose_cast(h_ts, ln_sb["ln1g"], ln_sb["ln1b"], hT)
        for dd in range(NDT):
            for w_sb, dst in ((wq_sb, qT_c[c]), (wk_sb, kT_c[c])):
                ps = ps_mm.tile([128, TC], f32, name="ps_qk", tag="mm")
                for k in range(NET):
                    nc.tensor.matmul(
                        ps, w_sb[:, k, dd * 128:(dd + 1) * 128],
                        hT[:, k, :],
                        start=(k == 0), stop=(k == NET - 1),
                    )
                nc.scalar.copy(dst[:, dd, :], ps)
        for s in range(NSUB):
            ps = ps_mm.tile([128, DSL], f32, name="ps_v", tag="mm")
            for k in range(NET):
                nc.tensor.matmul(
                    ps, hT[:, k, s * TS:(s + 1) * TS], wv_sb[:, k, :],
                    start=(k == 0), stop=(k == NET - 1),
                )
            nc.scalar.copy(
                vt_c[c][:, s, :, 0:HS],
                ps.rearrange("p (h d) -> p h d", h=HPC),
            )
            nc.gpsimd.memset(vt_c[c][:, s, :, HS:HS + 1], 1.0)

    # =====================================================================
    # Phase 2: attention + proj partials
    # =====================================================================
    def attention_chunk(c, fillers=()):
        """fillers: emitted one per head-pair iteration - independent PE work
        (proj subtiles of an earlier chunk, the first FFN region) that keeps
        the Tensor engine dense while Act paces the exp pipeline."""
        fillers = list(fillers)
        nkt = (c + 1) * NSUB
        attTs = []
        for pr in range(NDT):  # head pair = d-tile
            av_ps = [ps_av.tile([128, TC], f32, name="avp") for _ in range(2)]

            def do_scores(i):
                m = i - c * NSUB
                w0 = m * TS if m > 0 else 0  # suffix start (diagonal tiles)
                pts = []
                for hh in range(2):
                    h0 = hh * HS
                    sc = ps_sc.tile([TS, TC], f32, name="sc")
                    nc.tensor.matmul(
                        sc[:, w0:TC],
                        kT_c[i // NSUB][h0:h0 + HS, pr,
                                        (i % NSUB) * TS:(i % NSUB + 1) * TS],
                        qT_c[c][h0:h0 + HS, pr, w0:TC],
                        start=True, stop=True,
                    )
                    pt = pt_pool.tile([TS, TC], mdt, name="pt")
                    nc.scalar.activation(
                        out=pt[:, w0:TC], in_=sc[:, w0:TC], func=AF.Exp,
                        scale=SCALE,
                    )
                    if m >= 0:
                        # diagonal block: zero t_k > t_q inside [m*TS,(m+1)*TS)
                        nc.vector.tensor_mul(
                            pt[:, m * TS:(m + 1) * TS],
                            pt[:, m * TS:(m + 1) * TS], mask_sb,
                        )
                    pts.append((pt, w0))
                return pts

            def do_avs(i, pts):
                for hh in range(2):
                    pt, w0 = pts[hh]
                    nc.tensor.matmul(
                        av_ps[hh][0:HS + 1, w0:TC],
                        vt_c[i // NSUB][:, i % NSUB, pr * 2 + hh, :],
                        pt[:, w0:TC],
                        start=(i == 0), stop=(i == nkt - 1),
                        skip_group_check=True,
                    )

            # stagger: emit scores+exp for a group of 4 t_k tiles, then the
            # av matmuls of the previous group, so PE never waits on exp.
            G = 4
            prev = []
            for g0 in range(0, nkt, G):
                cur = [(i, do_scores(i)) for i in range(g0, min(g0 + G, nkt))]
                for i, pts in prev:
                    do_avs(i, pts)
                prev = cur
            for i, pts in prev:
                do_avs(i, pts)

            # --- denominators, fully on-chip: reciprocal of the PSUM ones-row
            # then a K=1 ones matmul broadcasts it into partitions 64:128 of
            # the same PSUM bank; one DVE multiply normalizes.
            attT = attT_pool.tile([128, TC], mdt, name="attT")
            for hh in range(2):
                av_sb = avs_pool.tile([HS + 1, TC], f32, name="av_sb")
                nc.vector.tensor_copy(av_sb, av_ps[hh][0:HS + 1, :])
                recb = rec_pool.tile([1, TC], mdt, name="recb")
                with nc.allow_low_precision(reason="bf16 softmax denominators"):
                    nc.vector.reciprocal(out=recb, in_=av_sb[HS:HS + 1, :])
                nc.tensor.matmul(
                    av_ps[hh][HS:HS + HS, :], ones64b, recb,
                    start=True, stop=True, skip_group_check=True,
                )
                nc.vector.tensor_mul(
                    attT[hh * HS:(hh + 1) * HS, :], av_sb[0:HS, :],
                    av_ps[hh][HS:HS + HS, :],
                )
            attTs.append(attT)
            if fillers:
                fillers.pop(0)()
        for f in fillers:
            f()
        return attTs

    def proj_subtile(c, attTs, s):
        # ar_in layout: region r = c%2 rows [r*2TC,(r+1)*2TC): rank-0 chunk
        # first then rank-1 chunk, so each ReduceScatter input is contiguous.
        r0 = c * TC + s * TS
        w0 = (c % 2) * (2 * TC) + (c // 2) * TC + s * TS
        xb_t = xb_pool.tile([128, E], f32, name="xb_t")
        nc.gpsimd.dma_start(out=xb_t, in_=xb[r0:r0 + TS, :])
        part = part_pool.tile([128, E], mdt, name="part")
        for n in range(2):
            ps = ps_mm.tile([128, TC], f32, name="ps_pr", tag="mm")
            for dd in range(NDT):
                nc.tensor.matmul(
                    ps, attTs[dd][:, s * TS:(s + 1) * TS],
                    wp_sb[:, dd, n * TC:(n + 1) * TC],
                    start=(dd == 0), stop=(dd == NDT - 1),
                )
            nc.vector.tensor_add(
                part[:, n * TC:(n + 1) * TC], ps, xb_t[:, n * TC:(n + 1) * TC]
            )
        nc.gpsimd.dma_start(out=ar_in[w0:w0 + TS, :], in_=part)

    def proj_chunk(c, attTs):
        for s in range(NSUB):
            proj_subtile(c, attTs, s)

    def proj_fillers(c, attTs):
        from functools import partial
        return [partial(proj_subtile, c, attTs, s) for s in range(NSUB)]

    # =====================================================================
    # Phase 3: LN2 + FFN + residual for one reduced region (one chunk/core)
    # =====================================================================
    def ffn_region(r):
        h2T = hT_pool.tile([ET, NET, TC], mdt, name="h2T")
        x2_ts = []
        h2_ts = []
        for s in range(NSUB):
            x2_t = x2_pool.tile([128, E], mdt, name="x2_t")
            nc.gpsimd.dma_start(out=x2_t, in_=rs_out[r][s * TS:(s + 1) * TS, :])
            x2_ts.append(x2_t)
            h2_ts.append(h_pool.tile([128, E], mdt, name="h2_t", tag="h_t"))
        layer_norm_chunk(x2_ts, eps2_sb, h2_ts)
        transpose_cast(h2_ts, ln_sb["ln2g"], ln_sb["ln2b"], h2T)
        f1 = f1_pool.tile([FFN + 1, TC], mdt, name="f1")
        nc.vector.memset(f1, 1.0)  # row FFN stays 1.0 (b2 matmul row)
        ps_f = ps_mm.tile([FFN, TC], f32, name="ps_f", tag="mm")
        for k in range(NET):
            nc.tensor.matmul(
                ps_f, w1_sb[:, k, :], h2T[:, k, :],
                start=(k == 0), stop=(k == NET - 1),
            )
        nc.scalar.activation(
            out=f1[0:FFN, :], in_=ps_f, func=AF.Relu, bias=b1_sb, scale=1.0
        )
        for s in range(NSUB):
            o_t = out_pool.tile([128, E], f32, name="o_t")
            for n in range(2):
                ps = ps_mm.tile([128, TC], f32, name="ps_o", tag="mm")
                nc.tensor.matmul(
                    ps, f1[:, s * TS:(s + 1) * TS],
                    w2_sb[:, n * TC:(n + 1) * TC],
                    start=True, stop=True,
                )
                nc.vector.tensor_add(
                    o_t[:, n * TC:(n + 1) * TC], ps,
                    x2_ts[s][:, n * TC:(n + 1) * TC],
                )
            r0 = r * TC + s * TS
            nc.gpsimd.dma_start(out=out[r0:r0 + TS, :], in_=o_t)

    # ---- schedule: att(0)/att(2) pulled into the QKV phase; proj subtiles
    # of completed chunks (and ffn_region(0)) interleave into later attention
    # chunks' head-pair loops as Tensor-engine filler ----
    ln_qkv_chunk(0)
    ln_qkv_chunk(1)
    ln_qkv_chunk(2)
    attTs0 = attention_chunk(0)
    ln_qkv_chunk(3)
    attTs2 = attention_chunk(2, proj_fillers(0, attTs0))
    attTs1 = attention_chunk(1, proj_fillers(2, attTs2))
    nc.gpsimd.collective_compute(
        "ReduceScatter", mybir.AluOpType.add, replica_groups=PAIRS,
        ins=[ar_in[0:2 * TC, :]], outs=[rs_out[0]],
    )
    attTs3 = attention_chunk(
        3, proj_fillers(1, attTs1) + [lambda: ffn_region(0)]
    )
    proj_chunk(3, attTs3)
    nc.gpsimd.collective_compute(
        "ReduceScatter", mybir.AluOpType.add, replica_groups=PAIRS,
        ins=[ar_in[2 * TC:4 * TC, :]], outs=[rs_out[1]],
    )
    ffn_region(1)


# =========================================================================
# Host side
# =========================================================================
_NC_CACHE = {}
RUN_KWARGS = {}      # test harness may set {"trace": True} for profiling
LAST_RESULT = None   # BassKernelResults of the most recent run


def kernel(x, wq, wk, wv, w_proj, b_proj, w1, b1, w2, b2, ln1_g, ln1_b, ln2_g,
           ln2_b):
    mode = MM_MODE
    np_mdt = _np_mdt(mode)
    if mode not in _NC_CACHE:
        _NC_CACHE[mode] = build(mode)
    nc = _NC_CACHE[mode]

    x = np.asarray(x, np.float32)
    # lower-triangle [128,128] block mask: valid (1.0) iff t_k(p) <= t_q(f)
    mask = np.tril(np.ones((TS, TS), np.float32)).T.astype(np_mdt)
    identity = np.eye(TS, dtype=np.float32).astype(np_mdt)
    w2e = np.concatenate([np.asarray(w2, np.float32),
                          np.asarray(b2, np.float32)[None, :]], axis=0)
    in_maps = []
    for core in range(NCORE):
        b, g = core // 2, core % 2
        sl = slice(g * DSL, (g + 1) * DSL)
        in_maps.append({
            "xr": 0.5 * x[b],
            "xb": 0.5 * x[b] + 0.5 * np.asarray(b_proj, np.float32)[None, :],
            "wq": np.asarray(wq, np.float32)[:, sl].astype(np_mdt),
            "wk": np.asarray(wk, np.float32)[:, sl].astype(np_mdt),
            "wv": np.asarray(wv, np.float32)[:, sl].astype(np_mdt),
            "wp": np.asarray(w_proj, np.float32)[sl, :].astype(np_mdt),
            "w1": np.asarray(w1, np.float32).astype(np_mdt),
            "w2e": w2e.astype(np_mdt),
            "b1": np.asarray(b1, np.float32)[:, None],
            "ln1g": np.asarray(ln1_g, np.float32)[:, None],
            "ln1b": np.asarray(ln1_b, np.float32)[:, None],
            "ln2g": np.asarray(ln2_g, np.float32)[:, None],
            "ln2b": np.asarray(ln2_b, np.float32)[:, None],
            "mask": mask,
            "ident": identity,
        })
    global LAST_RESULT
    res = run_bass_kernel_spmd(nc, in_maps, list(range(NCORE)), **RUN_KWARGS)
    LAST_RESULT = res
    outp = np.empty((B, T, E), np.float32)
    for core in range(NCORE):
        b, g = core // 2, core % 2
        outp[b, g * (T // 2):(g + 1) * (T // 2), :] = res.results[core]["out"]
    return outp


# revision 36
# speedup vs baseline: 1.4711x; 1.1954x over previous
"""Trainium2 Bass kernel: pre-LN transformer block (B=4, T=2048, E=1024, H=16, FFN=100).

Sharding (8 NeuronCores): core 2b+g handles batch b, head-group g (8 of 16 heads,
i.e. a 512-wide slice of the QKV output dim / proj input dim).  Both cores of a
pair compute attention + proj partials for all 2048 tokens of their batch; two
per-pair ReduceScatters (bf16, Shared outputs) combine the partials and hand
each core half the tokens, on which it runs LN2 + FFN and writes its
[1024, 1024] output shard.

SPMD notes: all 8 cores run one program; per-core behavior differs only via
input data.  The residual is fed as x/2 on both pair members (summed back to x
by the reduce); LN1 uses eps/4 so layernorm(x/2, eps/4) == layernorm(x, eps)
exactly.  b_proj/2 is folded host-side into the proj residual copy of x (xb),
and b2 is folded into the FFN second matmul as an extra input row.

Attention layout: scores are computed transposed, S^T[t_k, t_q] = k^T.T @ q^T,
with q^T/k^T in [head_dim, token] layout (from PE-transposed LN output, all in
bf16 so transposes run at 1 cycle/row).  Softmax runs without max subtraction
(logits are ~N(0, 0.25), safe in fp32): exp on ScalarE straight out of PSUM
with the 1/sqrt(E) scale folded in.  Causality is exploited at 128-column
granularity: for the diagonal t_k tile at offset m only the suffix columns
[m*128:) are computed/exp'd, and a single [128,128] lower-triangle mask fixes
the diagonal block.  The denominator comes from an extra ones-column appended
to V; its reciprocal row is broadcast across partitions 64:128 of the same AV
PSUM bank via a K=1 ones matmul (fully on-chip - no DRAM bounce), and one DVE
multiply produces the normalized attention output.
"""

from contextlib import ExitStack

import numpy as np
import ml_dtypes

import concourse.bass as bass
import concourse.mybir as mybir
import concourse.tile as tile
from concourse.bass_utils import run_bass_kernel_spmd
from concourse.vector_clock import ScopedClock


class SplitDrainTC(tile.TileContext):
    """Works around a walrus codegen limit: an SP CTRL instruction may carry
    only one sync wait, so the kernel-tail drain's waits are split onto
    preceding single-wait nops."""

    def _drain_and_barrier(self, tick_clock, wait_clock):
        probe = self.nc.sync.nop(nofuse=True)
        wait_clock.add_sem_waits(
            probe.ins, ScopedClock({None: tick_clock.global_clock})
        )
        si = probe.ins.sync_info
        waits = list(si.on_wait) if si is not None else []
        if len(waits) > 1:
            si.on_wait = [waits[0]]
            for w in waits[1:]:
                n2 = self.nc.sync.nop(nofuse=True)
                n2.ins.sync_info = mybir.SyncInfo(on_wait=[w], on_update=[])
        self.nc.sync.drain()
        self.nc.all_engine_barrier()
        popped = self.nc._tile_sem_poison_stack.pop()
        assert popped is self._sem_poison
        self.nc.clear_and_free_semaphores(list(self.sems.allocated().values()))
        self.nc.all_engine_barrier()

B, T, E, H, HS, FFN = 4, 2048, 1024, 16, 64, 100
EPS = 1e-5
NCORE = 8
TC = 512            # token chunk
NTC = T // TC       # 4
TS = 128            # token subtile
NSUB = TC // TS     # 4
ET = 128            # embed tile
NET = E // ET       # 8
DSL = E // 2        # per-core qkv output slice (8 heads * 64)
NDT = DSL // 128    # 4 d-tiles (2 heads each)
HPC = H // 2        # 8 heads per core
SCALE = float(E) ** -0.5
PAIRS = [[0, 1], [2, 3], [4, 5], [6, 7]]

MM_MODE = "bf16"    # "bf16" | "f32"
AF = mybir.ActivationFunctionType


def _mdt(mode):
    return mybir.dt.bfloat16 if mode == "bf16" else mybir.dt.float32


def _np_mdt(mode):
    return ml_dtypes.bfloat16 if mode == "bf16" else np.float32


def build(mode=MM_MODE):
    f32 = mybir.dt.float32
    mdt = _mdt(mode)

    nc = bass.Bass(num_devices=NCORE)

    io = {}

    def param(name, shape, dtype):
        io[name] = nc.declare_dram_parameter(name, shape, dtype, isOutput=False)

    param("xr", [T, E], mdt)           # x/2 (LN1 input)
    param("xb", [T, E], mdt)           # x/2 + b_proj/2 (proj residual)
    param("wq", [E, DSL], mdt)
    param("wk", [E, DSL], mdt)
    param("wv", [E, DSL], mdt)
    param("wp", [DSL, E], mdt)
    param("w1", [E, FFN], mdt)
    param("w2e", [FFN + 1, E], mdt)    # w2 with b2 as the extra last row
    param("b1", [FFN, 1], f32)
    param("ln1g", [E, 1], f32)
    param("ln1b", [E, 1], f32)
    param("ln2g", [E, 1], f32)
    param("ln2b", [E, 1], f32)
    param("mask", [TS, TS], mdt)       # lower-triangle (t_k <= t_q) block mask
    param("ident", [TS, TS], mdt)
    io["out"] = nc.declare_dram_parameter("out", [T // 2, E], f32, isOutput=True)

    with SplitDrainTC(nc) as tc:
        with ExitStack() as ctx:
            _build_tile(ctx, tc, nc, mode, mdt, f32, io)
    _split_waits(nc)
    return nc


def _split_waits(nc, maxw=1):
    """walrus codegen accepts a limited number of sync waits per instruction;
    move the excess onto same-engine NoOps inserted just before."""
    import bass_rust
    n = 0
    for f in nc.m.functions:
        for b in f.blocks:
            new = []
            for inst in b.instructions:
                si = inst.sync_info
                # fixed-length ISA instructions can't carry waits at all
                cap = 0 if isinstance(inst, bass_rust.InstISA) else maxw
                if si is not None and len(si.on_wait) > cap:
                    waits = list(si.on_wait)
                    keep = waits[-cap:] if cap else []
                    excess = waits[:-cap] if cap else waits
                    for w in excess:
                        nop = mybir.InstNoOp(
                            name=f"{inst.name}-wsplit{n}", engine=inst.engine
                        )
                        nop.bass_nofuse = True
                        n += 1
                        nop.sync_info = mybir.SyncInfo(
                            on_wait=[w], on_update=[]
                        )
                        new.append(nop)
                    si.on_wait = keep
                new.append(inst)
            if n:
                b.instructions = new


def _build_tile(ctx, tc, nc, mode, mdt, f32, io):
    xr, xb, out = io["xr"], io["xb"], io["out"]

    def pool(name, bufs, space="SBUF"):
        return ctx.enter_context(tc.tile_pool(name=name, bufs=bufs, space=space))

    # ---- internal DRAM ----
    dram = pool("dram", 1, space="DRAM")
    ar_in = dram.tile([T, E], mdt, name="ar_in")
    rs_out = [
        dram.tile([TC, E], mdt, name="rs0_out"),
        dram.tile([TC, E], mdt, name="rs1_out"),
    ]

    # ---- persistent SBUF: weights & constants (DMAs on the idle Pool
    # sequencer; its DGE config time is ~25ns vs SP's ~565ns) ----
    wpool = pool("weights", 1)
    wq_sb = wpool.tile([ET, NET, DSL], mdt, name="wq_sb")
    wk_sb = wpool.tile([ET, NET, DSL], mdt, name="wk_sb")
    wv_sb = wpool.tile([ET, NET, DSL], mdt, name="wv_sb")
    # big weights go on the (otherwise idle) SP queue so the gpsimd queue can
    # start streaming x tiles immediately
    nc.sync.dma_start(out=wq_sb, in_=io["wq"].rearrange("(k p) d -> p k d", p=ET))
    nc.sync.dma_start(out=wk_sb, in_=io["wk"].rearrange("(k p) d -> p k d", p=ET))
    nc.sync.dma_start(out=wv_sb, in_=io["wv"].rearrange("(k p) d -> p k d", p=ET))
    wp_sb = wpool.tile([128, NDT, E], mdt, name="wp_sb")
    nc.sync.dma_start(out=wp_sb, in_=io["wp"].rearrange("(k p) d -> p k d", p=128))
    ones64b = wpool.tile([1, HS], mdt, name="ones64b")
    nc.vector.memset(ones64b, 1.0)
    w1_sb = wpool.tile([ET, NET, FFN], mdt, name="w1_sb")
    nc.sync.dma_start(out=w1_sb, in_=io["w1"].rearrange("(k p) d -> p k d", p=ET))
    w2_sb = wpool.tile([FFN + 1, E], mdt, name="w2_sb")
    nc.sync.dma_start(out=w2_sb, in_=io["w2e"][:])
    b1_sb = wpool.tile([FFN, 1], f32, name="b1_sb")
    nc.sync.dma_start(out=b1_sb, in_=io["b1"][:])
    ln_sb = {}
    for nm in ("ln1g", "ln1b", "ln2g", "ln2b"):
        ln_sb[nm] = wpool.tile([ET, NET, 1], f32, name=nm + "_sb")
        nc.gpsimd.dma_start(
            out=ln_sb[nm], in_=io[nm].rearrange("(k p) o -> p k o", p=ET)
        )
    mask_sb = wpool.tile([TS, TS], mdt, name="mask_sb")
    nc.gpsimd.dma_start(out=mask_sb, in_=io["mask"][:])
    id_sb = wpool.tile([TS, TS], mdt, name="id_sb")
    nc.gpsimd.dma_start(out=id_sb, in_=io["ident"][:])
    eps1_sb = wpool.tile([128, 1], f32, name="eps1_sb")
    nc.vector.memset(eps1_sb, EPS / 4.0)  # LN1 runs on x/2
    eps2_sb = wpool.tile([128, 1], f32, name="eps2_sb")
    nc.vector.memset(eps2_sb, EPS)

    # ---- persistent SBUF: per-chunk K^T, V(+ones), Q^T ----
    kv = pool("kv", 1)
    kT_c = [kv.tile([128, NDT, TC], mdt, name=f"kT{c}") for c in range(NTC)]
    vt_c = [kv.tile([128, NSUB, HPC, HS + 1], mdt, name=f"vt{c}")
            for c in range(NTC)]
    qT_c = [kv.tile([128, NDT, TC], mdt, name=f"qT{c}") for c in range(NTC)]

    # ---- working pools ----
    xt_pool = pool("xt", 4)
    h_pool = pool("h", 5)
    mv_pool = pool("mv", 3)
    hT_pool = pool("hT", 2)
    pt_pool = pool("pt", 6)
    avs_pool = pool("avs", 4)
    rec_pool = pool("rec", 4)
    attT_pool = pool("attT", 8)   # att(0) and att(2) outputs both live
    xb_pool = pool("xbp", 2)
    part_pool = pool("part", 3)
    x2_pool = pool("x2", 4)
    f1_pool = pool("f1", 2)
    out_pool = pool("outp", 2)
    ps_mm = pool("ps_mm", 2, space="PSUM")
    ps_sc = pool("ps_sc", 2, space="PSUM")
    ps_av = pool("ps_av", 2, space="PSUM")

    def layer_norm_chunk(x_ts, eps_tile, out_ts):
        """out_ts[s] (bf16) = (x_ts[s] - mean) * rsqrt(var + eps), with the
        4 subtiles' stats batched so Sqrt costs one Act instruction (one
        activation-table region instead of four)."""
        n = len(x_ts)
        mv = mv_pool.tile([128, n, 2], f32, name="mv")
        for s, x_t in enumerate(x_ts):
            stats = mv_pool.tile(
                [128, 2, nc.vector.BN_STATS_DIM], f32, name="stats"
            )
            xg = x_t.rearrange("p (u q) -> p u q", u=2)
            for u in range(2):
                nc.vector.bn_stats(out=stats[:, u, :], in_=xg[:, u, :])
            nc.vector.bn_aggr(out=mv[:, s, :], in_=stats)
        rstd = mv_pool.tile([128, n], f32, name="rstd")
        nc.scalar.activation(
            out=rstd, in_=mv[:, :, 1], func=AF.Sqrt, bias=eps_tile, scale=1.0
        )
        nc.vector.reciprocal(out=rstd, in_=rstd)
        for s, x_t in enumerate(x_ts):
            nc.vector.tensor_scalar(
                out=out_ts[s], in0=x_t, scalar1=mv[:, s, 0:1],
                scalar2=rstd[:, s:s + 1],
                op0=mybir.AluOpType.subtract, op1=mybir.AluOpType.mult,
            )

    def transpose_cast(h_ts, g_sb, b_sb, hT):
        """PE-transpose 4 subtiles of h [128, E] (bf16) into hT[:, k, :],
        batching the 4 128x128 transposes of one k-tile into one PSUM tile so
        the layernorm scale/bias fold costs one DVE op per [128, 512]."""
        for k in range(NET):
            tp = ps_mm.tile([TS, TC], mdt, name="tp", tag="mm")
            for s in range(NSUB):
                nc.tensor.transpose(
                    tp[:, s * TS:(s + 1) * TS],
                    h_ts[s][:, k * ET:(k + 1) * ET], id_sb,
                )
            nc.vector.tensor_scalar(
                out=hT[:, k, :], in0=tp,
                scalar1=g_sb[:, k, :], scalar2=b_sb[:, k, :],
                op0=mybir.AluOpType.mult, op1=mybir.AluOpType.add,
            )

    # =====================================================================
    # Phase 1: LN1 + transpose + QKV per chunk
    # =====================================================================
    def ln_qkv_chunk(c):
        hT = hT_pool.tile([ET, NET, TC], mdt, name="hT")
        x_ts = []
        h_ts = []
        for s in range(NSUB):
            r0 = c * TC + s * TS
            x_t = xt_pool.tile([128, E], mdt, name="x_t")
            nc.gpsimd.dma_start(out=x_t, in_=xr[r0:r0 + TS, :])
            x_ts.append(x_t)
            h_ts.append(h_pool.tile([128, E], mdt, name="h_t"))
        layer_norm_chunk(x_ts, eps1_sb, h_ts)
        transpose_cast(h_ts, ln_sb["ln1g"], ln_sb["ln1b"], hT)
        for dd in range(NDT):
            for w_sb, dst in ((wq_sb, qT_c[c]), (wk_sb, kT_c[c])):
                ps = ps_mm.tile([128, TC], f32, name="ps_qk", tag="mm")
                for k in range(NET):
                    nc.tensor.matmul(
                        ps, w_sb[:, k, dd * 128:(dd + 1) * 128],
                        hT[:, k, :],
                        start=(k == 0), stop=(k == NET - 1),
                    )
                nc.scalar.copy(dst[:, dd, :], ps)
        for s in range(NSUB):
            ps = ps_mm.tile([128, DSL], f32, name="ps_v", tag="mm")
            for k in range(NET):
                nc.tensor.matmul(
                    ps, hT[:, k, s * TS:(s + 1) * TS], wv_sb[:, k, :],
                    start=(k == 0), stop=(k == NET - 1),
                )
            nc.scalar.copy(
                vt_c[c][:, s, :, 0:HS],
                ps.rearrange("p (h d) -> p h d", h=HPC),
            )
            nc.gpsimd.memset(vt_c[c][:, s, :, HS:HS + 1], 1.0)

    # =====================================================================
    # Phase 2: attention + proj partials
    # =====================================================================
    def attention_chunk(c, fillers=()):
        """fillers: emitted one per head-pair iteration - independent PE work
        (proj subtiles of an earlier chunk, the first FFN region) that keeps
        the Tensor engine dense while Act paces the exp pipeline."""
        fillers = list(fillers)
        nkt = (c + 1) * NSUB
        attTs = []
        for pr in range(NDT):  # head pair = d-tile
            av_ps = [ps_av.tile([128, TC], f32, name="avp") for _ in range(2)]

            def do_scores(i):
                m = i - c * NSUB
                w0 = m * TS if m > 0 else 0  # suffix start (diagonal tiles)
                sc = ps_sc.tile([TS, 2, TC], f32, name="sc")  # 2 PSUM banks
                for hh in range(2):
                    h0 = hh * HS
                    nc.tensor.matmul(
                        sc[:, hh, w0:TC],
                        kT_c[i // NSUB][h0:h0 + HS, pr,
                                        (i % NSUB) * TS:(i % NSUB + 1) * TS],
                        qT_c[c][h0:h0 + HS, pr, w0:TC],
                        start=True, stop=True,
                    )
                pt = pt_pool.tile([TS, 2, TC], mdt, name="pt")
                # one exp covers both heads' score tiles
                nc.scalar.activation(
                    out=pt[:, :, w0:TC], in_=sc[:, :, w0:TC], func=AF.Exp,
                    scale=SCALE,
                )
                if m >= 0:
                    # diagonal block: zero t_k > t_q inside [m*TS,(m+1)*TS)
                    for hh in range(2):
                        nc.vector.tensor_mul(
                            pt[:, hh, m * TS:(m + 1) * TS],
                            pt[:, hh, m * TS:(m + 1) * TS], mask_sb,
                        )
                return (pt, w0)

            def do_avs(i, ptw):
                pt, w0 = ptw
                for hh in range(2):
                    nc.tensor.matmul(
                        av_ps[hh][0:HS + 1, w0:TC],
                        vt_c[i // NSUB][:, i % NSUB, pr * 2 + hh, :],
                        pt[:, hh, w0:TC],
                        start=(i == 0), stop=(i == nkt - 1),
                        skip_group_check=True,
                    )

            # stagger: emit scores+exp for a group of 4 t_k tiles, then the
            # av matmuls of the previous group, so PE never waits on exp.
            G = 4
            prev = []
            for g0 in range(0, nkt, G):
                cur = [(i, do_scores(i)) for i in range(g0, min(g0 + G, nkt))]
                for i, pts in prev:
                    do_avs(i, pts)
                prev = cur
            for i, pts in prev:
                do_avs(i, pts)

            # --- denominators, fully on-chip: reciprocal of the PSUM ones-row
            # then a K=1 ones matmul broadcasts it into partitions 64:128 of
            # the same PSUM bank; one DVE multiply normalizes.
            attT = attT_pool.tile([128, TC], mdt, name="attT")
            for hh in range(2):
                av_sb = avs_pool.tile([HS + 1, TC], f32, name="av_sb")
                nc.vector.tensor_copy(av_sb, av_ps[hh][0:HS + 1, :])
                recb = rec_pool.tile([1, TC], mdt, name="recb")
                with nc.allow_low_precision(reason="bf16 softmax denominators"):
                    nc.vector.reciprocal(out=recb, in_=av_sb[HS:HS + 1, :])
                nc.tensor.matmul(
                    av_ps[hh][HS:HS + HS, :], ones64b, recb,
                    start=True, stop=True, skip_group_check=True,
                )
                nc.vector.tensor_mul(
                    attT[hh * HS:(hh + 1) * HS, :], av_sb[0:HS, :],
                    av_ps[hh][HS:HS + HS, :],
                )
            attTs.append(attT)
            if fillers:
                fillers.pop(0)()
        for f in fillers:
            f()
        return attTs

    def proj_subtile(c, attTs, s):
        # ar_in layout: region r = c%2 rows [r*2TC,(r+1)*2TC): rank-0 chunk
        # first then rank-1 chunk, so each ReduceScatter input is contiguous.
        r0 = c * TC + s * TS
        w0 = (c % 2) * (2 * TC) + (c // 2) * TC + s * TS
        xb_t = xb_pool.tile([128, E], mdt, name="xb_t")
        nc.gpsimd.dma_start(out=xb_t, in_=xb[r0:r0 + TS, :])
        part = part_pool.tile([128, E], mdt, name="part")
        for n in range(2):
            ps = ps_mm.tile([128, TC], f32, name="ps_pr", tag="mm")
            for dd in range(NDT):
                nc.tensor.matmul(
                    ps, attTs[dd][:, s * TS:(s + 1) * TS],
                    wp_sb[:, dd, n * TC:(n + 1) * TC],
                    start=(dd == 0), stop=(dd == NDT - 1),
                )
            nc.vector.tensor_add(
                part[:, n * TC:(n + 1) * TC], ps, xb_t[:, n * TC:(n + 1) * TC]
            )
        nc.gpsimd.dma_start(out=ar_in[w0:w0 + TS, :], in_=part)

    def proj_chunk(c, attTs):
        for s in range(NSUB):
            proj_subtile(c, attTs, s)

    def proj_fillers(c, attTs):
        from functools import partial
        return [partial(proj_subtile, c, attTs, s) for s in range(NSUB)]

    # =====================================================================
    # Phase 3: LN2 + FFN + residual for one reduced region (one chunk/core)
    # =====================================================================
    def ffn_region(r):
        h2T = hT_pool.tile([ET, NET, TC], mdt, name="h2T")
        x2_ts = []
        h2_ts = []
        for s in range(NSUB):
            x2_t = x2_pool.tile([128, E], mdt, name="x2_t")
            nc.gpsimd.dma_start(out=x2_t, in_=rs_out[r][s * TS:(s + 1) * TS, :])
            x2_ts.append(x2_t)
            h2_ts.append(h_pool.tile([128, E], mdt, name="h2_t", tag="h_t"))
        layer_norm_chunk(x2_ts, eps2_sb, h2_ts)
        transpose_cast(h2_ts, ln_sb["ln2g"], ln_sb["ln2b"], h2T)
        f1 = f1_pool.tile([FFN + 1, TC], mdt, name="f1")
        nc.vector.memset(f1, 1.0)  # row FFN stays 1.0 (b2 matmul row)
        ps_f = ps_mm.tile([FFN, TC], f32, name="ps_f", tag="mm")
        for k in range(NET):
            nc.tensor.matmul(
                ps_f, w1_sb[:, k, :], h2T[:, k, :],
                start=(k == 0), stop=(k == NET - 1),
            )
        nc.scalar.activation(
            out=f1[0:FFN, :], in_=ps_f, func=AF.Relu, bias=b1_sb, scale=1.0
        )
        for s in range(NSUB):
            o_t = out_pool.tile([128, E], f32, name="o_t")
            for n in range(2):
                ps = ps_mm.tile([128, TC], f32, name="ps_o", tag="mm")
                nc.tensor.matmul(
                    ps, f1[:, s * TS:(s + 1) * TS],
                    w2_sb[:, n * TC:(n + 1) * TC],
                    start=True, stop=True,
                )
                nc.vector.tensor_add(
                    o_t[:, n * TC:(n + 1) * TC], ps,
                    x2_ts[s][:, n * TC:(n + 1) * TC],
                )
            r0 = r * TC + s * TS
            nc.gpsimd.dma_start(out=out[r0:r0 + TS, :], in_=o_t)

    # ---- schedule: att(0)/att(2) pulled into the QKV phase; proj subtiles
    # of completed chunks (and ffn_region(0)) interleave into later attention
    # chunks' head-pair loops as Tensor-engine filler ----
    ln_qkv_chunk(0)
    ln_qkv_chunk(1)
    ln_qkv_chunk(2)
    attTs0 = attention_chunk(0)
    ln_qkv_chunk(3)
    attTs2 = attention_chunk(2, proj_fillers(0, attTs0))
    attTs1 = attention_chunk(1, proj_fillers(2, attTs2))
    nc.gpsimd.collective_compute(
        "ReduceScatter", mybir.AluOpType.add, replica_groups=PAIRS,
        ins=[ar_in[0:2 * TC, :]], outs=[rs_out[0]],
    )
    attTs3 = attention_chunk(
        3, proj_fillers(1, attTs1) + [lambda: ffn_region(0)]
    )
    proj_chunk(3, attTs3)
    nc.gpsimd.collective_compute(
        "ReduceScatter", mybir.AluOpType.add, replica_groups=PAIRS,
        ins=[ar_in[2 * TC:4 * TC, :]], outs=[rs_out[1]],
    )
    ffn_region(1)


# =========================================================================
# Host side
# =========================================================================
_NC_CACHE = {}
RUN_KWARGS = {}      # test harness may set {"trace": True} for profiling
LAST_RESULT = None   # BassKernelResults of the most recent run


def kernel(x, wq, wk, wv, w_proj, b_proj, w1, b1, w2, b2, ln1_g, ln1_b, ln2_g,
           ln2_b):
    mode = MM_MODE
    np_mdt = _np_mdt(mode)
    if mode not in _NC_CACHE:
        _NC_CACHE[mode] = build(mode)
    nc = _NC_CACHE[mode]

    x = np.asarray(x, np.float32)
    # lower-triangle [128,128] block mask: valid (1.0) iff t_k(p) <= t_q(f)
    mask = np.tril(np.ones((TS, TS), np.float32)).T.astype(np_mdt)
    identity = np.eye(TS, dtype=np.float32).astype(np_mdt)
    w2e = np.concatenate([np.asarray(w2, np.float32),
                          np.asarray(b2, np.float32)[None, :]], axis=0)
    in_maps = []
    for core in range(NCORE):
        b, g = core // 2, core % 2
        sl = slice(g * DSL, (g + 1) * DSL)
        in_maps.append({
            "xr": (0.5 * x[b]).astype(np_mdt),
            "xb": (0.5 * x[b] + 0.5 * np.asarray(b_proj, np.float32)[None, :]).astype(np_mdt),
            "wq": np.asarray(wq, np.float32)[:, sl].astype(np_mdt),
            "wk": np.asarray(wk, np.float32)[:, sl].astype(np_mdt),
            "wv": np.asarray(wv, np.float32)[:, sl].astype(np_mdt),
            "wp": np.asarray(w_proj, np.float32)[sl, :].astype(np_mdt),
            "w1": np.asarray(w1, np.float32).astype(np_mdt),
            "w2e": w2e.astype(np_mdt),
            "b1": np.asarray(b1, np.float32)[:, None],
            "ln1g": np.asarray(ln1_g, np.float32)[:, None],
            "ln1b": np.asarray(ln1_b, np.float32)[:, None],
            "ln2g": np.asarray(ln2_g, np.float32)[:, None],
            "ln2b": np.asarray(ln2_b, np.float32)[:, None],
            "mask": mask,
            "ident": identity,
        })
    global LAST_RESULT
    res = run_bass_kernel_spmd(nc, in_maps, list(range(NCORE)), **RUN_KWARGS)
    LAST_RESULT = res
    outp = np.empty((B, T, E), np.float32)
    for core in range(NCORE):
        b, g = core // 2, core % 2
        outp[b, g * (T // 2):(g + 1) * (T // 2), :] = res.results[core]["out"]
    return outp
